# revision 22
# baseline (speedup 1.0000x reference)
"""Biased multi-head attention on 8 Trainium2 NeuronCores.

Sharding: batch x head-group. Core c handles batch b = c//4 and heads
4*(c%4) .. 4*(c%4)+3 (4 of 16 heads). Q/K/V projections are column-sharded
over the core's heads, scores/softmax/AV are fully local per head, and the
output projection is row-sharded (each core contributes a partial [D, L]
that the host sums per batch).

Key-side compaction (the "sparse attention" lever): keys at padded
positions get score -1e4, i.e. softmax weight exp(-1e4) -> 0, so every
byte and flop spent on them is wasted. The host drops padded key
positions up front: x / attn_bias are compacted to the nk unpadded keys
(padded up to K' = ceil(nk/384)*384 slots; slack slots are re-masked with
-1e4). The result only differs from the reference by the clamped floor
weight exp(-20)/Z ~ 1e-9 the reference assigns to padded keys.

v7 design notes (~175us max-core at full clock vs v2's ~212):
  - All DRAM tensors are stored pre-blocked so every dma_start is 128
    descriptors of one large contiguous per-partition run (8-12KB each)
    instead of 1024-1536 x 1KB: descriptor issuance (DIRECT2D, ~15ns
    each) runs ON the issuing engine's sequencer, and the v2 layouts
    stalled the exp/score stream behind 10-27us issuance bursts.
  - AV matmul blocks of unit u-1 are interleaved BETWEEN unit u's score
    slots (after the pair-0 matmuls) instead of batched at unit end:
    the scores phase outruns the ACT exp stream (1.06us/pair-exp) and
    stalls on the 2-deep score-PSUM rotation; the AV block gives ACT
    ~1us of catch-up per slot.
  - ex2 = ex1 * exp(bias) as one merged [P, 4, 512] DVE op per kt.
  - Ramp: pair-0 q/k projections emitted first, unit 0 interleaves the
    remaining projections between its slots (pair-0 pass then pair-1
    pass); x0 loads on the SW-DGE group in parallel with wqkT on the
    HW group; 24 warm-up matmuls cover the load wait (HAM clock gate).
  - sync queue = consts + outT stores only (stores wait 3-5us on data,
    which must not head-of-line-block bias loads); bias rotates
    [scalar, gpsimd, gpsimd]; stores split in quarters; tail AV is
    head-major so evac/norm of pair 0 overlap pair 1's AV matmuls.

Measured dead ends (do not retry blindly): per-hi exps (either PSUM
layout) lose the score-pair row-tiling concurrency or the exp lead;
fp8 DoubleRow projections are 2x faster on PE but give ~6.6% rel err
(random-sign sums keep per-element quantization noise, it does not
average down) vs the 2e-2 gate; psS-parity padding via reader-less
tiles wedges the pool rotation; moving oproj copies off ACT or
rebalancing bias queues regresses. NOTE: the chip lottery matters -
runs land at 2.4GHz (MM N=512 p50=216ns) or P0-throttled 2.0GHz
(p50=259, ~+20% wall); compare kernels only within the same state.
"""

import os

import numpy as np

B, L, D, H = 2, 2048, 1024, 16
dh = D // H          # 64
NCORES = 8
HPC = 4              # heads per core
P = 128

_compiled = None     # (Kp, nc): compiled module and its key-slot count
LAST_RESULT = None   # BassKernelResults of the most recent run (for profiling)


def _build(Kp):
    from contextlib import ExitStack

    import concourse.bass as bass
    import concourse.tile as tile
    from concourse import bacc, mybir
    from concourse.bass import ds, ts

    f32 = mybir.dt.float32
    f16 = mybir.dt.float16
    Act = mybir.ActivationFunctionType
    KT = Kp // P          # 128-wide key chunks
    KT3 = Kp // 384       # 384-wide key units
    NU = 4 * KT3          # stream units total

    nc = bacc.Bacc("TRN2", target_bir_lowering=False, debug=False,
                   num_devices=NCORES)

    # blocked layouts: one contiguous per-partition run per load/store
    xT_d = nc.dram_tensor("xT", [4, P, 8, 512], f16, kind="ExternalInput").ap()
    xkT_d = nc.dram_tensor("xkT", [KT3, P, 8, 384], f16,
                           kind="ExternalInput").ap()
    wqkT_d = nc.dram_tensor("wqkT", [P, 8, 512], f16,
                            kind="ExternalInput").ap()
    wvT_d = nc.dram_tensor("wvT", [P, 8, 256], f16, kind="ExternalInput").ap()
    bqk_d = nc.dram_tensor("bqk", [P, 4], f32, kind="ExternalInput").ap()
    maskT_d = nc.dram_tensor("maskT", [P, KT], f32, kind="ExternalInput").ap()
    biasT_d = nc.dram_tensor("biasT", [NU, P, 3, HPC, 512], f16,
                             kind="ExternalInput").ap()
    woutT_d = nc.dram_tensor("woutT", [P, 2, D], f16,
                             kind="ExternalInput").ap()
    outT_d = nc.dram_tensor("outT", [4, P, 8, 512], f16,
                            kind="ExternalOutput").ap()

    with tile.TileContext(nc) as tc, ExitStack() as ctx:
        consts = ctx.enter_context(tc.tile_pool(name="consts", bufs=1))
        xp = ctx.enter_context(tc.tile_pool(name="xp", bufs=2))
        xkp = ctx.enter_context(tc.tile_pool(name="xkp", bufs=2))
        biasp = ctx.enter_context(tc.tile_pool(name="biasp", bufs=4))
        ex1p = ctx.enter_context(tc.tile_pool(name="ex1p", bufs=2))
        ex2p = ctx.enter_context(tc.tile_pool(name="ex2p", bufs=2))
        normp = ctx.enter_context(tc.tile_pool(name="normp", bufs=5))
        outp = ctx.enter_context(tc.tile_pool(name="outp", bufs=2))
        otlp = ctx.enter_context(tc.tile_pool(name="otlp", bufs=2))
        psS = ctx.enter_context(tc.tile_pool(name="psS", bufs=2, space="PSUM"))
        psAV = ctx.enter_context(tc.tile_pool(name="psAV", bufs=4, space="PSUM"))

        # criticality-ordered initial loads, split across both DGE groups:
        # HW (sync, scalar): wqkT + x0 + xk0 + small consts the first exps
        # and evacs need; SW (gpsimd): V-projection weights and the rest.
        wqkT_sb = consts.tile([P, 8, 512], f16, name="wqkT_sb", tag="wqkT_sb")
        nc.sync.dma_start(wqkT_sb, wqkT_d)

        xtiles = {}

        def qproj_dma(lt, eng=None):
            xs = xp.tile([P, 8, 512], f16, name="xs", tag="xs")
            (eng or nc.gpsimd).dma_start(xs, xT_d[lt])
            xtiles["q", lt] = xs

        def kvproj_dma(kt3, eng=None):
            xks = xkp.tile([P, 8, 384], f16, name="xks", tag="xks")
            (eng or nc.gpsimd).dma_start(xks, xkT_d[kt3])
            xtiles["kv", kt3] = xks

        qproj_dma(0)
        kvproj_dma(0, nc.scalar)
        maskT_sb = consts.tile([P, KT], f32, name="maskT_sb", tag="maskT_sb")
        nc.sync.dma_start(maskT_sb, maskT_d)
        bqk_sb = consts.tile([P, 4], f32, name="bqk_sb", tag="bqk_sb")
        nc.scalar.dma_start(bqk_sb, bqk_d)
        wvT_sb = consts.tile([P, 8, 256], f16, name="wvT_sb", tag="wvT_sb")
        nc.gpsimd.dma_start(wvT_sb, wvT_d)
        woutT_sb = consts.tile([P, 2, D], f16, name="woutT_sb", tag="woutT_sb")
        nc.gpsimd.dma_start(woutT_sb, woutT_d)

        qT_sb = consts.tile([P, 2, L], f16, name="qT_sb", tag="qT_sb")
        kT_sb = consts.tile([P, 2, Kp], f16, name="kT_sb", tag="kT_sb")
        V_sb = consts.tile([P, KT, HPC, 65], f16, name="V_sb", tag="V_sb")

        ones_sb = consts.tile([P, 64], f16, name="ones_sb", tag="ones_sb")
        nc.vector.memset(ones_sb, 1.0)
        nc.vector.tensor_copy(
            V_sb[:, :, :, 64:65],
            ones_sb[:, 0:1, None, None].to_broadcast((P, KT, HPC, 1)),
        )

        # HAM warm-up: ~9us of dependency-free matmuls at t=0 (covering the
        # initial DMA wait) so the PE clock gate is at full rate when the
        # first projection matmuls arrive.
        dummy_sb = consts.tile([P, 512], f16, name="dummy_sb", tag="dummy_sb")
        nc.vector.memset(dummy_sb, 0.5)
        sink_sb = consts.tile([P, 512], f16, name="sink_sb", tag="sink_sb")
        dps = psS.tile([P, 2, 512], f32, name="ps_s", tag="ps_s")
        for i in range(24):
            nc.tensor.matmul(dps[:, 0, :], lhsT=dummy_sb[:, 0:128],
                             rhs=dummy_sb, start=(i == 0), stop=(i == 23))
        nc.vector.tensor_copy(sink_sb, dps[:, 0, :])
        # pre-load the ACT exp table before the stream needs it
        nc.scalar.activation(sink_sb[:, 0:1], dummy_sb[:, 0:1],
                             Act.Exp, scale=1.0)

        # ---- projections (dma / mm chunks separated for scheduling) ----
        def qproj_mm(lt, rt):
            ps = psS.tile([P, 2, 512], f32, name="ps_s", tag="ps_s")
            for dc in range(8):
                nc.tensor.matmul(
                    ps[:, 0, :],
                    lhsT=wqkT_sb[:, dc, ts(rt, P)],
                    rhs=xtiles["q", lt][:, dc, :],
                    start=(dc == 0), stop=(dc == 7),
                )
            nc.vector.tensor_scalar_add(
                qT_sb[:, rt, ts(lt, 512)], ps[:, 0, :], bqk_sb[:, rt:rt + 1])

        def kproj_mm(kt3, rt):
            xks = xtiles["kv", kt3]
            ps = psS.tile([P, 2, 512], f32, name="ps_s", tag="ps_s")
            for dc in range(8):
                nc.tensor.matmul(
                    ps[:, 0, :384],
                    lhsT=wqkT_sb[:, dc, ts(2 + rt, P)],
                    rhs=xks[:, dc, :],
                    start=(dc == 0), stop=(dc == 7),
                )
            nc.vector.tensor_scalar_add(
                kT_sb[:, rt, ts(kt3, 384)], ps[:, 0, :384],
                bqk_sb[:, 2 + rt:3 + rt])

        def vproj_mm(kt3, l4):
            xks = xtiles["kv", kt3]
            ltv = kt3 * 3 + l4
            psv = psS.tile([P, 2, 512], f32, name="ps_s", tag="ps_s")
            for dc in range(8):
                nc.tensor.matmul(
                    psv[:, 0, :256],
                    lhsT=xks[:, dc, ts(l4, P)],
                    rhs=wvT_sb[:, dc, :],
                    start=(dc == 0), stop=(dc == 7),
                )
            nc.vector.tensor_copy(
                V_sb[:, ltv, :, 0:64],
                psv[:, 0, :256].rearrange("p (h c) -> p h c", c=64),
            )

        # ---- attention stream ------------------------------------------
        avs = {}        # qt -> [4] PSUM accumulators [65, 512]
        avcs = {}       # qt -> [4] SBUF f16 copies
        ex1s = {}       # u -> merged [P, 3, 4, 512] exp tile
        ex2s = {}       # u -> merged [P, 3, 4, 512] weight tile
        btws = {}       # u -> bias tile
        otls = {}       # qt -> [P, 2, 512] f16 normalized O_T

        def ka_mm(ps):
            # cheap N=128 garbage matmul overwritten by the real producer
            # (start=True resets the accumulation); exists to deny the HAM
            # clock gate a contiguous idle window (keeps the PE at full rate).
            nc.tensor.matmul(ps[:, 0:128], lhsT=dummy_sb[:, 0:128],
                             rhs=dummy_sb[:, 0:128], start=True, stop=True)

        def emit_scores(qt, kt, pair, ka=0):
            swide = psS.tile([P, 2, 512], f32, name="ps_s", tag="ps_s")
            for _ in range(ka):
                ka_mm(swide[:, 0, :])
            for hi in range(2):
                cs = slice(64 * hi, 64 * hi + 64)
                nc.tensor.matmul(
                    swide[:, hi, :],
                    lhsT=kT_sb[cs, pair, ts(kt, P)],
                    rhs=qT_sb[cs, pair, ts(qt, 512)],
                    start=True, stop=True,
                )
            nc.scalar.activation(
                ex1s["cur"][:, kt % 3, ds(2 * pair, 2), :], swide, Act.Exp,
                bias=maskT_sb[:, kt:kt + 1], scale=1.0)

        def emit_mult(u, kt):
            # ex2 = ex1 * exp(bias): one merged op per kt (both pairs)
            nc.vector.tensor_mul(
                ex2s[u][:, kt % 3, :, :],
                ex1s["cur"][:, kt % 3, :, :],
                btws[u][:, kt % 3, :, :])

        def emit_av_block(w, j):
            wqt, wkt3 = divmod(w, KT3)
            if wkt3 == 0 and j == 0:
                avs[wqt] = [psAV.tile([65, 512], f32, name=f"av{wqt}{h}",
                                      tag="av") for h in range(4)]
            kt = 3 * wkt3 + j
            for h in range(4):
                nc.tensor.matmul(
                    avs[wqt][h],
                    lhsT=V_sb[:, kt, h, :],
                    rhs=ex2s[w][:, j, h, :],
                    start=(kt == 0), stop=(kt == KT - 1),
                )
            if j == 2:
                ex2s.pop(w)

        def emit_evac(qt, split=False):
            avcs[qt] = []
            for h in range(4):
                avc = normp.tile([65, 512], f16, name="avc", tag="avc")
                if split and h < 2:
                    nc.scalar.copy(avc, avs[qt][h])
                else:
                    nc.vector.tensor_copy(avc, avs[qt][h])
                avcs[qt].append(avc)

        def norm_cc(qt, cc):
            # heads {2cc, 2cc+1} -> otl[:, cc]; even head on partitions
            # 0:64, odd shifted to 64:128 via SBUF->SBUF DMA.
            e, o = avcs[qt][2 * cc], avcs[qt][2 * cc + 1]
            otl = otls[qt]
            zb = psS.tile([P, 2, 512], f32, name="ps_s", tag="ps_s")
            ka_mm(zb[:, 0, :])
            ka_mm(zb[:, 0, :])
            nc.tensor.matmul(zb[0:64, 0, :], lhsT=ones_sb[64:65, :],
                             rhs=e[64:65, :], start=True, stop=True)
            nc.tensor.matmul(zb[64:128, 0, :], lhsT=ones_sb[64:65, :],
                             rhs=o[64:65, :], start=True, stop=True)
            zr = normp.tile([P, 512], f32, name="zr", tag="zr")
            nc.vector.reciprocal_approx_fast(zr, zb[:, 0, :])
            avsh = normp.tile([P, 512], f16, name="avsh", tag="avsh")
            nc.gpsimd.dma_start(avsh[64:128, :], o[0:64, :])
            nc.vector.tensor_mul(otl[0:64, cc, :], e[0:64, :], zr[0:64, :])
            nc.vector.tensor_mul(otl[64:128, cc, :], avsh[64:128, :],
                                 zr[64:128, :])

        def oproj_mm(qt, jt, ka=0):
            ps = psS.tile([P, 2, 512], f32, name="ps_s", tag="ps_s")
            for _ in range(ka):
                ka_mm(ps[:, 0, :])
            for cc in range(2):
                nc.tensor.matmul(
                    ps[:, 0, :],
                    lhsT=woutT_sb[:, cc, ts(jt, P)],
                    rhs=otls[qt][:, cc, :],
                    start=(cc == 0), stop=(cc == 1),
                )
            if jt % 2 == 0:
                nc.scalar.copy(osbs[qt][:, jt, :], ps[:, 0, :])
            else:
                nc.vector.tensor_copy(osbs[qt][:, jt, :], ps[:, 0, :])

        osbs = {}

        def oproj_store(qt, half, quarters=2):
            w = 4 // quarters
            for q4 in range(quarters):
                o0 = 4 * half + w * q4
                nc.sync.dma_start(
                    outT_d[qt][:, ds(o0, w), :],
                    osbs[qt][:, ds(o0, w), :])

        # ---- filler schedule -------------------------------------------
        # fillers[u] = list of (dma_fn | None, [mm_fn, ...]); dma issued one
        # unit ahead, mm chunks interleaved between the unit's score slots.
        def F(dma, *mms):
            return (dma, list(mms))

        def mk_norm(qt, cc):
            def go():
                if cc == 0:
                    otls[qt] = otlp.tile([P, 2, 512], f16, name="otl",
                                         tag="otl")
                norm_cc(qt, cc)
            return go

        fillers = {u: [] for u in range(NU)}
        for kt3 in range(1, KT3):
            # kv projection for unit kt3 runs as filler in unit kt3-1
            fillers[kt3 - 1].append(F(
                (lambda k=kt3: kvproj_dma(k)),
                (lambda k=kt3: kproj_mm(k, 0)),
                (lambda k=kt3: kproj_mm(k, 1)),
                (lambda k=kt3: vproj_mm(k, 0)),
                (lambda k=kt3: vproj_mm(k, 1)),
                (lambda k=kt3: vproj_mm(k, 2)),
            ))
        for qt in range(1, 4):
            # q projection for block qt: qproj(1) late in stream 0, later
            # ones inside the (PE-thin) boundary units of streams 1 and 2
            u = 2 if qt == 1 else (qt - 1) * KT3
            fillers[u].append(F(
                (lambda l=qt: qproj_dma(l)),
                (lambda l=qt: qproj_mm(l, 0)),
                (lambda l=qt: qproj_mm(l, 1)),
            ))
        def mk_oproj(q, h):
            def go():
                if h == 0:
                    osbs[q] = outp.tile([P, 8, 512], f16, name="osb",
                                        tag="osb")
                for jt in range(4 * h, 4 * h + 4):
                    oproj_mm(q, jt, ka=1 if jt % 4 == 0 else 0)
                oproj_store(q, h)
            return go

        for qt in range(3):
            # norm of qt in the second unit of stream qt+1 (the evac is
            # emitted at the qt boundary, right after the last AV block);
            # out-proj follows in the same / next unit.
            u0 = (qt + 1) * KT3
            fillers[u0 + 1].append(F(
                None,
                mk_norm(qt, 0),
                mk_norm(qt, 1),
                mk_oproj(qt, 0),
            ))
            fillers[u0 + min(2, KT3 - 1)].append(F(None, mk_oproj(qt, 1)))

        # ---- emission ---------------------------------------------------
        # pair-0 enablers first so unit 0's scores start right after them
        qproj_mm(0, 0)
        kproj_mm(0, 0)

        for dma_fn, _ in fillers[0]:
            if dma_fn is not None:
                dma_fn()
        btw_engs = [nc.scalar, nc.gpsimd, nc.gpsimd]
        prev = None
        for u in range(NU):
            qt, kt3 = divmod(u, KT3)
            btw = biasp.tile([P, 3, HPC, 512], f16, name="btw", tag="btw")
            btw_engs[u % 3].dma_start(btw, biasT_d[u])
            btws[u] = btw
            if u + 1 < NU:
                for dma_fn, _ in fillers[u + 1]:
                    if dma_fn is not None:
                        dma_fn()
            chunks = []
            for _, mms in fillers[u]:
                chunks.extend(mms)
            ex1s["cur"] = ex1p.tile([P, 3, 4, 512], f16, name="ex1",
                                    tag="ex1")
            ex2s[u] = ex2p.tile([P, 3, 4, 512], f16, name="ex2", tag="ex2")
            if u == 0:
                # unit 0: pair-0 pass then pair-1 pass, remaining
                # projections interleaved between the slots
                u0c = [lambda: qproj_mm(0, 1), lambda: kproj_mm(0, 1),
                       lambda: vproj_mm(0, 0), lambda: vproj_mm(0, 1),
                       lambda: vproj_mm(0, 2)] + chunks
                si = 0
                for pair in range(2):
                    for kt in range(3):
                        emit_scores(0, kt, pair, ka=1 if pair == 0 else 0)
                        take = -(-(len(u0c) - si) // (6 - pair * 3 - kt))
                        for _ in range(take):
                            u0c[si]()
                            si += 1
                for kt in range(3):
                    emit_mult(0, kt)
                btws.pop(0)
                prev = 0
                continue
            pqt, pkt3 = divmod(prev, KT3)
            boundary = (kt3 == 0)
            ci = 0
            for j, kt in enumerate(range(3 * kt3, 3 * kt3 + 3)):
                ka = 1
                if j == 0 and (boundary or u in (1, 2)):
                    ka = 2
                emit_scores(qt, kt, 0, ka=ka)
                emit_av_block(prev, j)
                emit_scores(qt, kt, 1)
                if j == 2 and pkt3 == KT3 - 1:
                    emit_evac(pqt)
                emit_mult(u, kt)
                take = -(-(len(chunks) - ci) // (3 - j))
                for _ in range(take):
                    chunks[ci]()
                    ci += 1
            btws.pop(u)
            prev = u

        tps = psS.tile([P, 2, 512], f32, name="ps_s", tag="ps_s")
        for _ in range(2):
            ka_mm(tps[:, 0, :])
        nc.vector.tensor_copy(sink_sb[:, 0:128], tps[:, 0, 0:128])
        # tail: head-major AV so evac/norm of pair 0 overlaps pair 1's AV
        avcs[3] = [None] * 4
        otls[3] = otlp.tile([P, 2, 512], f16, name="otl", tag="otl")
        osbs[3] = outp.tile([P, 8, 512], f16, name="osb", tag="osb")
        for h in range(4):
            for j in range(3):
                kt = 3 * (KT3 - 1) + j
                nc.tensor.matmul(
                    avs[3][h],
                    lhsT=V_sb[:, kt, h, :],
                    rhs=ex2s[prev][:, j, h, :],
                    start=(kt == 0), stop=(kt == KT - 1),
                )
            avc = normp.tile([65, 512], f16, name="avc", tag="avc")
            if h < 2:
                nc.scalar.copy(avc, avs[3][h])
            else:
                nc.vector.tensor_copy(avc, avs[3][h])
            avcs[3][h] = avc
            if h == 1:
                norm_cc(3, 0)
            elif h == 3:
                norm_cc(3, 1)
        ex2s.pop(prev)
        for half in range(2):
            for jt in range(4 * half, 4 * half + 4):
                oproj_mm(3, jt, ka=1)
            oproj_store(3, half)

    nc.compile()
    return nc


def _blk(a, inner):
    """[R, C] -> [C//inner, 128, R//128, inner] device-blocked layout:
    out[ct, p, o, i] = a[o*128 + p, ct*inner + i]."""
    R, C = a.shape
    return np.ascontiguousarray(
        a.reshape(R // P, P, C // inner, inner).transpose(2, 1, 0, 3))


def _prep_core_inputs(c, Kp, x, key_padding_mask, attn_bias, W_in, b_in,
                      W_out, b_out):
    b, hg = c // HPC, c % HPC
    hs = slice(256 * hg, 256 * hg + 256)
    f16 = np.float16
    KT3 = Kp // 384
    idx = np.where(~key_padding_mask[b])[0]
    nk = len(idx)
    wq, wk = W_in[0:D][hs], W_in[D:2 * D][hs]
    wv = W_in[2 * D:3 * D][hs]

    xk = np.zeros((Kp, D), dtype=np.float32)
    xk[:nk] = x[b][idx]
    # -ln(64) headroom shift: softmax is scale-invariant, and scaling all
    # exp weights by 1/64 keeps exp(s)*exp(bias) inside fp16 range.
    maskT = np.full(Kp, -10000.0, dtype=np.float32)
    maskT[:nk] = -np.log(64.0)
    eb = np.zeros((HPC, Kp, L), dtype=f16)
    eb[:, :nk, :] = np.exp(
        attn_bias[b, HPC * hg:HPC * hg + HPC][:, :, idx]
        .transpose(0, 2, 1)).astype(f16)
    # device layout [u, p, ktm, h, q]: u = qt*KT3 + kt3,
    # key slot k = (kt3*3 + ktm)*128 + p, query l = qt*512 + q
    biasT = np.ascontiguousarray(
        eb.reshape(HPC, KT3, 3, P, 4, 512)
        .transpose(4, 1, 3, 2, 0, 5)).reshape(4 * KT3, P, 3, HPC, 512)

    xT = np.ascontiguousarray(x[b].T, dtype=f16)       # [D, L]
    xkT = np.ascontiguousarray(xk.T, dtype=f16)        # [D, Kp]
    wqkT = np.concatenate([wq / 8.0, wk], 0).T.astype(f16)   # [D, 512]
    wvT = np.ascontiguousarray(wv.T, dtype=f16)        # [D, 256]
    woutT = np.ascontiguousarray(W_out[:, hs].T, dtype=f16)  # [256, D]
    maskT_blk = np.ascontiguousarray(
        maskT.reshape(Kp // P, P).T.astype(np.float32))
    bqk = np.concatenate([b_in[0:D][hs] / 8.0,
                          b_in[D:2 * D][hs]]).astype(np.float32)
    bqk_blk = np.ascontiguousarray(bqk.reshape(4, P).T)

    return {
        "xT": _blk(xT, 512),
        "xkT": _blk(xkT, 384),
        "wqkT": _blk(wqkT, 512)[0],
        "wvT": _blk(wvT, 256)[0],
        "bqk": bqk_blk,
        "maskT": maskT_blk,
        "biasT": biasT,
        "woutT": _blk(woutT, D)[0],
    }


def kernel(x, key_padding_mask, attn_bias, W_in, b_in, W_out, b_out):
    global _compiled, LAST_RESULT
    from concourse.bass_utils import run_bass_kernel_spmd

    nk_max = int((~key_padding_mask).sum(axis=1).max())
    Kp = max(384, -(-nk_max // 384) * 384)

    if _compiled is None or _compiled[0] != Kp:
        _compiled = (Kp, _build(Kp))

    in_maps = [
        _prep_core_inputs(c, Kp, x, key_padding_mask, attn_bias, W_in, b_in,
                          W_out, b_out)
        for c in range(NCORES)
    ]
    res = run_bass_kernel_spmd(
        _compiled[1], in_maps, core_ids=list(range(NCORES)),
        trace_cores=(list(range(NCORES))
                     if os.environ.get("BASS_TRACE") == "1" else None),
    )
    LAST_RESULT = res

    # host-side epilogue: sum row-sharded partials, add biases.
    bv = b_in[2 * D:3 * D].astype(np.float64)
    const = b_out.astype(np.float64) + W_out.astype(np.float64) @ bv  # [D]
    out = np.empty((B, L, D), dtype=np.float32)
    for b in range(B):
        acc = res.results[b * HPC]["outT"].astype(np.float64)
        for g in range(1, HPC):
            acc = acc + res.results[b * HPC + g]["outT"]
        # outT blocked [qt, p, o, q] -> [D, L] -> [L, D]
        full = acc.transpose(2, 1, 0, 3).reshape(D, L)
        out[b] = (full.T + const).astype(np.float32)
    return out


# revision 23
# speedup vs baseline: 1.0599x; 1.0599x over previous
"""Biased multi-head attention on 8 Trainium2 NeuronCores.

Sharding: batch x head-group. Core c handles batch b = c//4 and heads
4*(c%4) .. 4*(c%4)+3 (4 of 16 heads). Q/K/V projections are column-sharded
over the core's heads, scores/softmax/AV are fully local per head, and the
output projection is row-sharded (each core contributes a partial [D, L]
that the host sums per batch).

Key-side compaction (the "sparse attention" lever): keys at padded
positions get score -1e4, i.e. softmax weight exp(-1e4) -> 0, so every
byte and flop spent on them is wasted. The host drops padded key
positions up front: x / attn_bias are compacted to the nk unpadded keys
(padded up to K' = ceil(nk/384)*384 slots; slack slots are re-masked with
-1e4). The result only differs from the reference by the clamped floor
weight exp(-20)/Z ~ 1e-9 the reference assigns to padded keys.

v7 design notes (~175us max-core at full clock vs v2's ~212):
  - All DRAM tensors are stored pre-blocked so every dma_start is 128
    descriptors of one large contiguous per-partition run (8-12KB each)
    instead of 1024-1536 x 1KB: descriptor issuance (DIRECT2D, ~15ns
    each) runs ON the issuing engine's sequencer, and the v2 layouts
    stalled the exp/score stream behind 10-27us issuance bursts.
  - AV matmul blocks of unit u-1 are interleaved BETWEEN unit u's score
    slots (after the pair-0 matmuls) instead of batched at unit end:
    the scores phase outruns the ACT exp stream (1.06us/pair-exp) and
    stalls on the 2-deep score-PSUM rotation; the AV block gives ACT
    ~1us of catch-up per slot.
  - ex2 = ex1 * exp(bias) as one merged [P, 4, 512] DVE op per kt.
  - Ramp: pair-0 q/k projections emitted first, unit 0 interleaves the
    remaining projections between its slots (pair-0 pass then pair-1
    pass); x0 loads on the SW-DGE group in parallel with wqkT on the
    HW group; 24 warm-up matmuls cover the load wait (HAM clock gate).
  - sync queue = consts + outT stores only (stores wait 3-5us on data,
    which must not head-of-line-block bias loads); bias rotates
    [scalar, gpsimd, gpsimd]; stores split in quarters; tail AV is
    head-major so evac/norm of pair 0 overlap pair 1's AV matmuls.

Measured dead ends (do not retry blindly): per-hi exps (either PSUM
layout) lose the score-pair row-tiling concurrency or the exp lead;
fp8 DoubleRow projections are 2x faster on PE but give ~6.6% rel err
(random-sign sums keep per-element quantization noise, it does not
average down) vs the 2e-2 gate; psS-parity padding via reader-less
tiles wedges the pool rotation; moving oproj copies off ACT or
rebalancing bias queues regresses. NOTE: the chip lottery matters -
runs land at 2.4GHz (MM N=512 p50=216ns) or P0-throttled 2.0GHz
(p50=259, ~+20% wall); compare kernels only within the same state.
"""

import os

import numpy as np

B, L, D, H = 2, 2048, 1024, 16
dh = D // H          # 64
NCORES = 8
HPC = 4              # heads per core
P = 128

_compiled = None     # (Kp, nc): compiled module and its key-slot count
LAST_RESULT = None   # BassKernelResults of the most recent run (for profiling)


def _build(Kp):
    from contextlib import ExitStack

    import concourse.bass as bass
    import concourse.tile as tile
    from concourse import bacc, mybir
    from concourse.bass import ds, ts

    f32 = mybir.dt.float32
    f16 = mybir.dt.float16
    Act = mybir.ActivationFunctionType
    KT = Kp // P          # 128-wide key chunks
    KT3 = Kp // 384       # 384-wide key units
    NU = 4 * KT3          # stream units total

    nc = bacc.Bacc("TRN2", target_bir_lowering=False, debug=False,
                   num_devices=NCORES)

    # blocked layouts: one contiguous per-partition run per load/store
    xT_d = nc.dram_tensor("xT", [4, P, 8, 512], f16, kind="ExternalInput").ap()
    xkT_d = nc.dram_tensor("xkT", [KT3, P, 8, 384], f16,
                           kind="ExternalInput").ap()
    wqkT_d = nc.dram_tensor("wqkT", [P, 8, 512], f16,
                            kind="ExternalInput").ap()
    wvT_d = nc.dram_tensor("wvT", [P, 8, 256], f16, kind="ExternalInput").ap()
    bqk_d = nc.dram_tensor("bqk", [P, 4], f32, kind="ExternalInput").ap()
    maskT_d = nc.dram_tensor("maskT", [P, KT], f32, kind="ExternalInput").ap()
    biasT_d = nc.dram_tensor("biasT", [NU, P, 3, HPC, 512], f16,
                             kind="ExternalInput").ap()
    woutT_d = nc.dram_tensor("woutT", [P, 2, D], f16,
                             kind="ExternalInput").ap()
    outT_d = nc.dram_tensor("outT", [4, P, 8, 512], f16,
                            kind="ExternalOutput").ap()

    with tile.TileContext(nc) as tc, ExitStack() as ctx:
        consts = ctx.enter_context(tc.tile_pool(name="consts", bufs=1))
        xp = ctx.enter_context(tc.tile_pool(name="xp", bufs=2))
        xkp = ctx.enter_context(tc.tile_pool(name="xkp", bufs=2))
        biasp = ctx.enter_context(tc.tile_pool(name="biasp", bufs=3))
        ex1p = ctx.enter_context(tc.tile_pool(name="ex1p", bufs=2))
        ex2p = ctx.enter_context(tc.tile_pool(name="ex2p", bufs=2))
        normp = ctx.enter_context(tc.tile_pool(name="normp", bufs=5))
        outp = ctx.enter_context(tc.tile_pool(name="outp", bufs=2))
        otlp = ctx.enter_context(tc.tile_pool(name="otlp", bufs=2))
        psS = ctx.enter_context(tc.tile_pool(name="psS", bufs=2, space="PSUM"))
        psAV = ctx.enter_context(tc.tile_pool(name="psAV", bufs=4, space="PSUM"))

        # criticality-ordered initial loads, split across both DGE groups:
        # HW (sync, scalar): wqkT + x0 + xk0 + small consts the first exps
        # and evacs need; SW (gpsimd): V-projection weights and the rest.
        wqkT_sb = consts.tile([P, 8, 512], f16, name="wqkT_sb", tag="wqkT_sb")
        nc.sync.dma_start(wqkT_sb, wqkT_d)

        xtiles = {}

        def qproj_dma(lt, eng=None):
            xs = xp.tile([P, 8, 512], f16, name="xs", tag="xs")
            (eng or nc.gpsimd).dma_start(xs, xT_d[lt])
            xtiles["q", lt] = xs

        def kvproj_dma(kt3, eng=None):
            xks = xkp.tile([P, 8, 384], f16, name="xks", tag="xks")
            (eng or nc.gpsimd).dma_start(xks, xkT_d[kt3])
            xtiles["kv", kt3] = xks

        qproj_dma(0)
        kvproj_dma(0, nc.scalar)
        maskT_sb = consts.tile([P, KT], f32, name="maskT_sb", tag="maskT_sb")
        nc.sync.dma_start(maskT_sb, maskT_d)
        bqk_sb = consts.tile([P, 4], f32, name="bqk_sb", tag="bqk_sb")
        nc.scalar.dma_start(bqk_sb, bqk_d)
        wvT_sb = consts.tile([P, 8, 256], f16, name="wvT_sb", tag="wvT_sb")
        nc.gpsimd.dma_start(wvT_sb, wvT_d)
        woutT_sb = consts.tile([P, 2, D], f16, name="woutT_sb", tag="woutT_sb")
        nc.gpsimd.dma_start(woutT_sb, woutT_d)

        qT_sb = consts.tile([P, 2, L], f16, name="qT_sb", tag="qT_sb")
        kT_sb = consts.tile([P, 2, Kp], f16, name="kT_sb", tag="kT_sb")
        V_sb = consts.tile([P, KT, HPC, 65], f16, name="V_sb", tag="V_sb")

        ones_sb = consts.tile([P, 64], f16, name="ones_sb", tag="ones_sb")
        nc.vector.memset(ones_sb, 1.0)
        nc.vector.tensor_copy(
            V_sb[:, :, :, 64:65],
            ones_sb[:, 0:1, None, None].to_broadcast((P, KT, HPC, 1)),
        )

        # HAM warm-up: ~9us of dependency-free matmuls at t=0 (covering the
        # initial DMA wait) so the PE clock gate is at full rate when the
        # first projection matmuls arrive.
        dummy_sb = consts.tile([P, 512], f16, name="dummy_sb", tag="dummy_sb")
        nc.vector.memset(dummy_sb, 0.5)
        sink_sb = consts.tile([P, 512], f16, name="sink_sb", tag="sink_sb")
        dps = psS.tile([P, 2, 512], f32, name="ps_s", tag="ps_s")
        for i in range(24):
            nc.tensor.matmul(dps[:, 0, :], lhsT=dummy_sb[:, 0:128],
                             rhs=dummy_sb, start=(i == 0), stop=(i == 23))
        nc.vector.tensor_copy(sink_sb, dps[:, 0, :])
        # pre-load the ACT exp table before the stream needs it
        nc.scalar.activation(sink_sb[:, 0:1], dummy_sb[:, 0:1],
                             Act.Exp, scale=1.0)

        # ---- projections (dma / mm chunks separated for scheduling) ----
        def qproj_mm(lt, rt):
            ps = psS.tile([P, 2, 512], f32, name="ps_s", tag="ps_s")
            for dc in range(8):
                nc.tensor.matmul(
                    ps[:, 0, :],
                    lhsT=wqkT_sb[:, dc, ts(rt, P)],
                    rhs=xtiles["q", lt][:, dc, :],
                    start=(dc == 0), stop=(dc == 7),
                )
            nc.vector.tensor_scalar_add(
                qT_sb[:, rt, ts(lt, 512)], ps[:, 0, :], bqk_sb[:, rt:rt + 1])

        def kproj_mm(kt3, rt):
            xks = xtiles["kv", kt3]
            ps = psS.tile([P, 2, 512], f32, name="ps_s", tag="ps_s")
            for dc in range(8):
                nc.tensor.matmul(
                    ps[:, 0, :384],
                    lhsT=wqkT_sb[:, dc, ts(2 + rt, P)],
                    rhs=xks[:, dc, :],
                    start=(dc == 0), stop=(dc == 7),
                )
            nc.vector.tensor_scalar_add(
                kT_sb[:, rt, ts(kt3, 384)], ps[:, 0, :384],
                bqk_sb[:, 2 + rt:3 + rt])

        def vproj_mm(kt3, l4):
            xks = xtiles["kv", kt3]
            ltv = kt3 * 3 + l4
            psv = psS.tile([P, 2, 512], f32, name="ps_s", tag="ps_s")
            for dc in range(8):
                nc.tensor.matmul(
                    psv[:, 0, :256],
                    lhsT=xks[:, dc, ts(l4, P)],
                    rhs=wvT_sb[:, dc, :],
                    start=(dc == 0), stop=(dc == 7),
                )
            nc.vector.tensor_copy(
                V_sb[:, ltv, :, 0:64],
                psv[:, 0, :256].rearrange("p (h c) -> p h c", c=64),
            )

        # ---- attention stream ------------------------------------------
        avs = {}        # qt -> [4] PSUM accumulators [65, 512]
        avcs = {}       # qt -> [4] SBUF f16 copies
        ex1s = {}       # u -> merged [P, 3, 4, 512] exp tile
        ex2s = {}       # u -> merged [P, 3, 4, 512] weight tile
        btws = {}       # u -> bias tile
        otls = {}       # qt -> [P, 2, 512] f16 normalized O_T

        def ka_mm(ps):
            # cheap N=128 garbage matmul overwritten by the real producer
            # (start=True resets the accumulation); exists to deny the HAM
            # clock gate a contiguous idle window (keeps the PE at full rate).
            nc.tensor.matmul(ps[:, 0:128], lhsT=dummy_sb[:, 0:128],
                             rhs=dummy_sb[:, 0:128], start=True, stop=True)

        def emit_scores(qt, kt, pair, ka=0):
            swide = psS.tile([P, 2, 512], f32, name="ps_s", tag="ps_s")
            for _ in range(ka):
                ka_mm(swide[:, 0, :])
            for hi in range(2):
                cs = slice(64 * hi, 64 * hi + 64)
                nc.tensor.matmul(
                    swide[:, hi, :],
                    lhsT=kT_sb[cs, pair, ts(kt, P)],
                    rhs=qT_sb[cs, pair, ts(qt, 512)],
                    start=True, stop=True,
                )
            nc.scalar.activation(
                ex1s["cur"][:, kt % 3, ds(2 * pair, 2), :], swide, Act.Exp,
                bias=maskT_sb[:, kt:kt + 1], scale=1.0)

        def emit_mult(u, kt):
            # ex2 = ex1 * exp(bias): one merged op per kt (both pairs)
            nc.vector.tensor_mul(
                ex2s[u][:, kt % 3, :, :],
                ex1s["cur"][:, kt % 3, :, :],
                btws[u][:, kt % 3, :, :])

        def emit_av_block(w, j):
            wqt, wkt3 = divmod(w, KT3)
            if wkt3 == 0 and j == 0:
                avs[wqt] = [psAV.tile([65, 512], f32, name=f"av{wqt}{h}",
                                      tag="av") for h in range(4)]
            kt = 3 * wkt3 + j
            for h in range(4):
                nc.tensor.matmul(
                    avs[wqt][h],
                    lhsT=V_sb[:, kt, h, :],
                    rhs=ex2s[w][:, j, h, :],
                    start=(kt == 0), stop=(kt == KT - 1),
                )
            if j == 2:
                ex2s.pop(w)

        def emit_evac(qt, split=False):
            avcs[qt] = []
            for h in range(4):
                avc = normp.tile([65, 512], f16, name="avc", tag="avc")
                if split and h < 2:
                    nc.scalar.copy(avc, avs[qt][h])
                else:
                    nc.vector.tensor_copy(avc, avs[qt][h])
                avcs[qt].append(avc)

        def norm_cc(qt, cc):
            # heads {2cc, 2cc+1} -> otl[:, cc]; even head on partitions
            # 0:64, odd shifted to 64:128 via SBUF->SBUF DMA.
            e, o = avcs[qt][2 * cc], avcs[qt][2 * cc + 1]
            otl = otls[qt]
            zb = psS.tile([P, 2, 512], f32, name="ps_s", tag="ps_s")
            ka_mm(zb[:, 0, :])
            ka_mm(zb[:, 0, :])
            nc.tensor.matmul(zb[0:64, 0, :], lhsT=ones_sb[64:65, :],
                             rhs=e[64:65, :], start=True, stop=True)
            nc.tensor.matmul(zb[64:128, 0, :], lhsT=ones_sb[64:65, :],
                             rhs=o[64:65, :], start=True, stop=True)
            zr = normp.tile([P, 512], f32, name="zr", tag="zr")
            nc.vector.reciprocal_approx_fast(zr, zb[:, 0, :])
            avsh = normp.tile([P, 512], f16, name="avsh", tag="avsh")
            nc.gpsimd.dma_start(avsh[64:128, :], o[0:64, :])
            nc.vector.tensor_mul(otl[0:64, cc, :], e[0:64, :], zr[0:64, :])
            nc.vector.tensor_mul(otl[64:128, cc, :], avsh[64:128, :],
                                 zr[64:128, :])

        def oproj_mm(qt, jt, ka=0):
            ps = psS.tile([P, 2, 512], f32, name="ps_s", tag="ps_s")
            for _ in range(ka):
                ka_mm(ps[:, 0, :])
            for cc in range(2):
                nc.tensor.matmul(
                    ps[:, 0, :],
                    lhsT=woutT_sb[:, cc, ts(jt, P)],
                    rhs=otls[qt][:, cc, :],
                    start=(cc == 0), stop=(cc == 1),
                )
            if jt % 2 == 0:
                nc.scalar.copy(osbs[qt][:, jt, :], ps[:, 0, :])
            else:
                nc.vector.tensor_copy(osbs[qt][:, jt, :], ps[:, 0, :])

        osbs = {}

        def oproj_store(qt, half, quarters=2):
            w = 4 // quarters
            for q4 in range(quarters):
                o0 = 4 * half + w * q4
                nc.sync.dma_start(
                    outT_d[qt][:, ds(o0, w), :],
                    osbs[qt][:, ds(o0, w), :])

        # ---- filler schedule -------------------------------------------
        # fillers[u] = list of (dma_fn | None, [mm_fn, ...]); dma issued one
        # unit ahead, mm chunks interleaved between the unit's score slots.
        def F(dma, *mms):
            return (dma, list(mms))

        def mk_norm(qt, cc):
            def go():
                if cc == 0:
                    otls[qt] = otlp.tile([P, 2, 512], f16, name="otl",
                                         tag="otl")
                norm_cc(qt, cc)
            return go

        fillers = {u: [] for u in range(NU)}
        for kt3 in range(1, KT3):
            # kv projection for unit kt3 runs as filler in unit kt3-1
            fillers[kt3 - 1].append(F(
                (lambda k=kt3: kvproj_dma(k)),
                (lambda k=kt3: kproj_mm(k, 0)),
                (lambda k=kt3: kproj_mm(k, 1)),
                (lambda k=kt3: vproj_mm(k, 0)),
                (lambda k=kt3: vproj_mm(k, 1)),
                (lambda k=kt3: vproj_mm(k, 2)),
            ))
        for qt in range(1, 4):
            # q projection for block qt: qproj(1) late in stream 0, later
            # ones inside the (PE-thin) boundary units of streams 1 and 2
            u = 2 if qt == 1 else (qt - 1) * KT3
            fillers[u].append(F(
                (lambda l=qt: qproj_dma(l)),
                (lambda l=qt: qproj_mm(l, 0)),
                (lambda l=qt: qproj_mm(l, 1)),
            ))
        def mk_oproj(q, h):
            def go():
                if h == 0:
                    osbs[q] = outp.tile([P, 8, 512], f16, name="osb",
                                        tag="osb")
                for jt in range(4 * h, 4 * h + 4):
                    oproj_mm(q, jt, ka=1 if jt % 4 == 0 else 0)
                oproj_store(q, h)
            return go

        for qt in range(3):
            # norm of qt in the second unit of stream qt+1 (the evac is
            # emitted at the qt boundary, right after the last AV block);
            # out-proj follows in the same / next unit.
            u0 = (qt + 1) * KT3
            fillers[u0 + 1].append(F(
                None,
                mk_norm(qt, 0),
                mk_norm(qt, 1),
                mk_oproj(qt, 0),
            ))
            fillers[u0 + min(2, KT3 - 1)].append(F(None, mk_oproj(qt, 1)))

        # ---- emission ---------------------------------------------------
        # pair-0 enablers first so unit 0's scores start right after them
        qproj_mm(0, 0)
        kproj_mm(0, 0)

        for dma_fn, _ in fillers[0]:
            if dma_fn is not None:
                dma_fn()
        btw_engs = [nc.scalar, nc.gpsimd, nc.gpsimd]
        prev = None
        for u in range(NU):
            qt, kt3 = divmod(u, KT3)
            btw = biasp.tile([P, 3, HPC, 512], f16, name="btw", tag="btw")
            btw_engs[u % 3].dma_start(btw, biasT_d[u])
            btws[u] = btw
            if u + 1 < NU:
                for dma_fn, _ in fillers[u + 1]:
                    if dma_fn is not None:
                        dma_fn()
            chunks = []
            for _, mms in fillers[u]:
                chunks.extend(mms)
            ex1s["cur"] = ex1p.tile([P, 3, 4, 512], f16, name="ex1",
                                    tag="ex1")
            ex2s[u] = ex2p.tile([P, 3, 4, 512], f16, name="ex2", tag="ex2")
            if u == 0:
                # unit 0: pair-0 pass then pair-1 pass, remaining
                # projections interleaved between the slots
                u0c = [lambda: qproj_mm(0, 1), lambda: kproj_mm(0, 1),
                       lambda: vproj_mm(0, 0), lambda: vproj_mm(0, 1),
                       lambda: vproj_mm(0, 2)] + chunks
                si = 0
                for pair in range(2):
                    for kt in range(3):
                        emit_scores(0, kt, pair, ka=1 if pair == 0 else 0)
                        take = -(-(len(u0c) - si) // (6 - pair * 3 - kt))
                        for _ in range(take):
                            u0c[si]()
                            si += 1
                for kt in range(3):
                    emit_mult(0, kt)
                btws.pop(0)
                prev = 0
                continue
            pqt, pkt3 = divmod(prev, KT3)
            boundary = (kt3 == 0)
            ci = 0
            for j, kt in enumerate(range(3 * kt3, 3 * kt3 + 3)):
                ka = 1
                if j == 0 and (boundary or u in (1, 2)):
                    ka = 2
                emit_scores(qt, kt, 0, ka=ka)
                emit_av_block(prev, j)
                emit_scores(qt, kt, 1)
                if j == 2 and pkt3 == KT3 - 1:
                    emit_evac(pqt)
                emit_mult(u, kt)
                take = -(-(len(chunks) - ci) // (3 - j))
                for _ in range(take):
                    chunks[ci]()
                    ci += 1
            btws.pop(u)
            prev = u

        tps = psS.tile([P, 2, 512], f32, name="ps_s", tag="ps_s")
        for _ in range(2):
            ka_mm(tps[:, 0, :])
        nc.vector.tensor_copy(sink_sb[:, 0:128], tps[:, 0, 0:128])
        # tail: head-major AV so evac/norm of pair 0 overlaps pair 1's AV
        avcs[3] = [None] * 4
        otls[3] = otlp.tile([P, 2, 512], f16, name="otl", tag="otl")
        osbs[3] = outp.tile([P, 8, 512], f16, name="osb", tag="osb")
        for h in range(4):
            for j in range(3):
                kt = 3 * (KT3 - 1) + j
                nc.tensor.matmul(
                    avs[3][h],
                    lhsT=V_sb[:, kt, h, :],
                    rhs=ex2s[prev][:, j, h, :],
                    start=(kt == 0), stop=(kt == KT - 1),
                )
            avc = normp.tile([65, 512], f16, name="avc", tag="avc")
            if h < 2:
                nc.scalar.copy(avc, avs[3][h])
            else:
                nc.vector.tensor_copy(avc, avs[3][h])
            avcs[3][h] = avc
            if h == 1:
                norm_cc(3, 0)
            elif h == 3:
                norm_cc(3, 1)
        ex2s.pop(prev)
        for half in range(2):
            for jt in range(4 * half, 4 * half + 4):
                oproj_mm(3, jt, ka=1)
            oproj_store(3, half)

    nc.compile()
    return nc


def _blk(a, inner):
    """[R, C] -> [C//inner, 128, R//128, inner] device-blocked layout:
    out[ct, p, o, i] = a[o*128 + p, ct*inner + i]."""
    R, C = a.shape
    return np.ascontiguousarray(
        a.reshape(R // P, P, C // inner, inner).transpose(2, 1, 0, 3))


def _prep_core_inputs(c, Kp, x, key_padding_mask, attn_bias, W_in, b_in,
                      W_out, b_out):
    b, hg = c // HPC, c % HPC
    hs = slice(256 * hg, 256 * hg + 256)
    f16 = np.float16
    KT3 = Kp // 384
    idx = np.where(~key_padding_mask[b])[0]
    nk = len(idx)
    wq, wk = W_in[0:D][hs], W_in[D:2 * D][hs]
    wv = W_in[2 * D:3 * D][hs]

    xk = np.zeros((Kp, D), dtype=np.float32)
    xk[:nk] = x[b][idx]
    # -ln(64) headroom shift: softmax is scale-invariant, and scaling all
    # exp weights by 1/64 keeps exp(s)*exp(bias) inside fp16 range.
    maskT = np.full(Kp, -10000.0, dtype=np.float32)
    maskT[:nk] = -np.log(64.0)
    eb = np.zeros((HPC, Kp, L), dtype=f16)
    eb[:, :nk, :] = np.exp(
        attn_bias[b, HPC * hg:HPC * hg + HPC][:, :, idx]
        .transpose(0, 2, 1)).astype(f16)
    # device layout [u, p, ktm, h, q]: u = qt*KT3 + kt3,
    # key slot k = (kt3*3 + ktm)*128 + p, query l = qt*512 + q
    biasT = np.ascontiguousarray(
        eb.reshape(HPC, KT3, 3, P, 4, 512)
        .transpose(4, 1, 3, 2, 0, 5)).reshape(4 * KT3, P, 3, HPC, 512)

    xT = np.ascontiguousarray(x[b].T, dtype=f16)       # [D, L]
    xkT = np.ascontiguousarray(xk.T, dtype=f16)        # [D, Kp]
    wqkT = np.concatenate([wq / 8.0, wk], 0).T.astype(f16)   # [D, 512]
    wvT = np.ascontiguousarray(wv.T, dtype=f16)        # [D, 256]
    woutT = np.ascontiguousarray(W_out[:, hs].T, dtype=f16)  # [256, D]
    maskT_blk = np.ascontiguousarray(
        maskT.reshape(Kp // P, P).T.astype(np.float32))
    bqk = np.concatenate([b_in[0:D][hs] / 8.0,
                          b_in[D:2 * D][hs]]).astype(np.float32)
    bqk_blk = np.ascontiguousarray(bqk.reshape(4, P).T)

    return {
        "xT": _blk(xT, 512),
        "xkT": _blk(xkT, 384),
        "wqkT": _blk(wqkT, 512)[0],
        "wvT": _blk(wvT, 256)[0],
        "bqk": bqk_blk,
        "maskT": maskT_blk,
        "biasT": biasT,
        "woutT": _blk(woutT, D)[0],
    }


def kernel(x, key_padding_mask, attn_bias, W_in, b_in, W_out, b_out):
    global _compiled, LAST_RESULT
    from concourse.bass_utils import run_bass_kernel_spmd

    nk_max = int((~key_padding_mask).sum(axis=1).max())
    Kp = max(384, -(-nk_max // 384) * 384)

    if _compiled is None or _compiled[0] != Kp:
        _compiled = (Kp, _build(Kp))

    in_maps = [
        _prep_core_inputs(c, Kp, x, key_padding_mask, attn_bias, W_in, b_in,
                          W_out, b_out)
        for c in range(NCORES)
    ]
    res = run_bass_kernel_spmd(
        _compiled[1], in_maps, core_ids=list(range(NCORES)),
        trace_cores=(list(range(NCORES))
                     if os.environ.get("BASS_TRACE") == "1" else None),
    )
    LAST_RESULT = res

    # host-side epilogue: sum row-sharded partials, add biases.
    bv = b_in[2 * D:3 * D].astype(np.float64)
    const = b_out.astype(np.float64) + W_out.astype(np.float64) @ bv  # [D]
    out = np.empty((B, L, D), dtype=np.float32)
    for b in range(B):
        acc = res.results[b * HPC]["outT"].astype(np.float64)
        for g in range(1, HPC):
            acc = acc + res.results[b * HPC + g]["outT"]
        # outT blocked [qt, p, o, q] -> [D, L] -> [L, D]
        full = acc.transpose(2, 1, 0, 3).reshape(D, L)
        out[b] = (full.T + const).astype(np.float32)
    return out


# revision 24
# speedup vs baseline: 1.0615x; 1.0015x over previous
"""Biased multi-head attention on 8 Trainium2 NeuronCores.

Sharding: batch x head-group. Core c handles batch b = c//4 and heads
4*(c%4) .. 4*(c%4)+3 (4 of 16 heads). Q/K/V projections are column-sharded
over the core's heads, scores/softmax/AV are fully local per head, and the
output projection is row-sharded (each core contributes a partial [D, L]
that the host sums per batch).

Key-side compaction (the "sparse attention" lever): keys at padded
positions get score -1e4, i.e. softmax weight exp(-1e4) -> 0, so every
byte and flop spent on them is wasted. The host drops padded key
positions up front: x / attn_bias are compacted to the nk unpadded keys
(padded up to K' = ceil(nk/384)*384 slots; slack slots are re-masked with
-1e4). The result only differs from the reference by the clamped floor
weight exp(-20)/Z ~ 1e-9 the reference assigns to padded keys.

v7 design notes (~175us max-core at full clock vs v2's ~212):
  - All DRAM tensors are stored pre-blocked so every dma_start is 128
    descriptors of one large contiguous per-partition run (8-12KB each)
    instead of 1024-1536 x 1KB: descriptor issuance (DIRECT2D, ~15ns
    each) runs ON the issuing engine's sequencer, and the v2 layouts
    stalled the exp/score stream behind 10-27us issuance bursts.
  - AV matmul blocks of unit u-1 are interleaved BETWEEN unit u's score
    slots (after the pair-0 matmuls) instead of batched at unit end:
    the scores phase outruns the ACT exp stream (1.06us/pair-exp) and
    stalls on the 2-deep score-PSUM rotation; the AV block gives ACT
    ~1us of catch-up per slot.
  - ex2 = ex1 * exp(bias) as one merged [P, 4, 512] DVE op per kt.
  - Ramp: pair-0 q/k projections emitted first, unit 0 interleaves the
    remaining projections between its slots (pair-0 pass then pair-1
    pass); x0 loads on the SW-DGE group in parallel with wqkT on the
    HW group; 24 warm-up matmuls cover the load wait (HAM clock gate).
  - sync queue = consts + outT stores only (stores wait 3-5us on data,
    which must not head-of-line-block bias loads); bias rotates
    [scalar, gpsimd, gpsimd]; stores split in quarters; tail AV is
    head-major so evac/norm of pair 0 overlap pair 1's AV matmuls.

Measured dead ends (do not retry blindly): per-hi exps (either PSUM
layout) lose the score-pair row-tiling concurrency or the exp lead;
fp8 DoubleRow projections are 2x faster on PE but give ~6.6% rel err
(random-sign sums keep per-element quantization noise, it does not
average down) vs the 2e-2 gate; psS-parity padding via reader-less
tiles wedges the pool rotation; moving oproj copies off ACT or
rebalancing bias queues regresses. NOTE: the chip lottery matters -
runs land at 2.4GHz (MM N=512 p50=216ns) or P0-throttled 2.0GHz
(p50=259, ~+20% wall); compare kernels only within the same state.
"""

import os

import numpy as np

B, L, D, H = 2, 2048, 1024, 16
dh = D // H          # 64
NCORES = 8
HPC = 4              # heads per core
P = 128

_compiled = None     # (Kp, nc): compiled module and its key-slot count
LAST_RESULT = None   # BassKernelResults of the most recent run (for profiling)


def _build(Kp):
    from contextlib import ExitStack

    import concourse.bass as bass
    import concourse.tile as tile
    from concourse import bacc, mybir
    from concourse.bass import ds, ts

    f32 = mybir.dt.float32
    f16 = mybir.dt.float16
    Act = mybir.ActivationFunctionType
    KT = Kp // P          # 128-wide key chunks
    KT3 = Kp // 384       # 384-wide key units
    NU = 4 * KT3          # stream units total

    nc = bacc.Bacc("TRN2", target_bir_lowering=False, debug=False,
                   num_devices=NCORES)

    # blocked layouts: one contiguous per-partition run per load/store
    xT_d = nc.dram_tensor("xT", [4, P, 8, 512], f16, kind="ExternalInput").ap()
    xkT_d = nc.dram_tensor("xkT", [KT3, P, 8, 384], f16,
                           kind="ExternalInput").ap()
    wqkT_d = nc.dram_tensor("wqkT", [P, 8, 512], f16,
                            kind="ExternalInput").ap()
    wvT_d = nc.dram_tensor("wvT", [P, 8, 256], f16, kind="ExternalInput").ap()
    bqk_d = nc.dram_tensor("bqk", [P, 4], f32, kind="ExternalInput").ap()
    maskT_d = nc.dram_tensor("maskT", [P, KT], f32, kind="ExternalInput").ap()
    biasT_d = nc.dram_tensor("biasT", [NU, P, 3, HPC, 512], f16,
                             kind="ExternalInput").ap()
    woutT_d = nc.dram_tensor("woutT", [P, 2, D], f16,
                             kind="ExternalInput").ap()
    outT_d = nc.dram_tensor("outT", [4, P, 8, 512], f16,
                            kind="ExternalOutput").ap()

    with tile.TileContext(nc) as tc, ExitStack() as ctx:
        consts = ctx.enter_context(tc.tile_pool(name="consts", bufs=1))
        xp = ctx.enter_context(tc.tile_pool(name="xp", bufs=2))
        xkp = ctx.enter_context(tc.tile_pool(name="xkp", bufs=2))
        biasp = ctx.enter_context(tc.tile_pool(name="biasp", bufs=3))
        ex1p = ctx.enter_context(tc.tile_pool(name="ex1p", bufs=2))
        ex2p = ctx.enter_context(tc.tile_pool(name="ex2p", bufs=2))
        normp = ctx.enter_context(tc.tile_pool(name="normp", bufs=5))
        outp = ctx.enter_context(tc.tile_pool(name="outp", bufs=2))
        otlp = ctx.enter_context(tc.tile_pool(name="otlp", bufs=2))
        psS = ctx.enter_context(tc.tile_pool(name="psS", bufs=2, space="PSUM"))
        psAV = ctx.enter_context(tc.tile_pool(name="psAV", bufs=4, space="PSUM"))

        # criticality-ordered initial loads, split across both DGE groups:
        # HW (sync, scalar): wqkT + x0 + xk0 + small consts the first exps
        # and evacs need; SW (gpsimd): V-projection weights and the rest.
        wqkT_sb = consts.tile([P, 8, 512], f16, name="wqkT_sb", tag="wqkT_sb")
        nc.sync.dma_start(wqkT_sb, wqkT_d)

        xtiles = {}

        def qproj_dma(lt, eng=None):
            xs = xp.tile([P, 8, 512], f16, name="xs", tag="xs")
            (eng or nc.gpsimd).dma_start(xs, xT_d[lt])
            xtiles["q", lt] = xs

        def kvproj_dma(kt3, eng=None):
            xks = xkp.tile([P, 8, 384], f16, name="xks", tag="xks")
            (eng or nc.gpsimd).dma_start(xks, xkT_d[kt3])
            xtiles["kv", kt3] = xks

        qproj_dma(0)
        kvproj_dma(0, nc.scalar)
        maskT_sb = consts.tile([P, KT], f32, name="maskT_sb", tag="maskT_sb")
        nc.sync.dma_start(maskT_sb, maskT_d)
        bqk_sb = consts.tile([P, 4], f32, name="bqk_sb", tag="bqk_sb")
        nc.scalar.dma_start(bqk_sb, bqk_d)
        wvT_sb = consts.tile([P, 8, 256], f16, name="wvT_sb", tag="wvT_sb")
        nc.gpsimd.dma_start(wvT_sb, wvT_d)
        woutT_sb = consts.tile([P, 2, D], f16, name="woutT_sb", tag="woutT_sb")
        nc.gpsimd.dma_start(woutT_sb, woutT_d)

        qT_sb = consts.tile([P, 2, L], f16, name="qT_sb", tag="qT_sb")
        kT_sb = consts.tile([P, 2, Kp], f16, name="kT_sb", tag="kT_sb")
        V_sb = consts.tile([P, KT, HPC, 65], f16, name="V_sb", tag="V_sb")

        ones_sb = consts.tile([P, 64], f16, name="ones_sb", tag="ones_sb")
        nc.vector.memset(ones_sb, 1.0)
        nc.vector.tensor_copy(
            V_sb[:, :, :, 64:65],
            ones_sb[:, 0:1, None, None].to_broadcast((P, KT, HPC, 1)),
        )

        # HAM warm-up: ~9us of dependency-free matmuls at t=0 (covering the
        # initial DMA wait) so the PE clock gate is at full rate when the
        # first projection matmuls arrive.
        dummy_sb = consts.tile([P, 512], f16, name="dummy_sb", tag="dummy_sb")
        nc.vector.memset(dummy_sb, 0.5)
        sink_sb = consts.tile([P, 512], f16, name="sink_sb", tag="sink_sb")
        dps = psS.tile([P, 2, 512], f32, name="ps_s", tag="ps_s")
        for i in range(36):
            nc.tensor.matmul(dps[:, 0, :], lhsT=dummy_sb[:, 0:128],
                             rhs=dummy_sb, start=(i == 0), stop=(i == 35))
        nc.vector.tensor_copy(sink_sb, dps[:, 0, :])
        # pre-load the ACT exp table before the stream needs it
        nc.scalar.activation(sink_sb[:, 0:1], dummy_sb[:, 0:1],
                             Act.Exp, scale=1.0)

        # ---- projections (dma / mm chunks separated for scheduling) ----
        def qproj_mm(lt, rt):
            ps = psS.tile([P, 2, 512], f32, name="ps_s", tag="ps_s")
            for dc in range(8):
                nc.tensor.matmul(
                    ps[:, 0, :],
                    lhsT=wqkT_sb[:, dc, ts(rt, P)],
                    rhs=xtiles["q", lt][:, dc, :],
                    start=(dc == 0), stop=(dc == 7),
                )
            nc.vector.tensor_scalar_add(
                qT_sb[:, rt, ts(lt, 512)], ps[:, 0, :], bqk_sb[:, rt:rt + 1])

        def kproj_mm(kt3, rt):
            xks = xtiles["kv", kt3]
            ps = psS.tile([P, 2, 512], f32, name="ps_s", tag="ps_s")
            for dc in range(8):
                nc.tensor.matmul(
                    ps[:, 0, :384],
                    lhsT=wqkT_sb[:, dc, ts(2 + rt, P)],
                    rhs=xks[:, dc, :],
                    start=(dc == 0), stop=(dc == 7),
                )
            nc.vector.tensor_scalar_add(
                kT_sb[:, rt, ts(kt3, 384)], ps[:, 0, :384],
                bqk_sb[:, 2 + rt:3 + rt])

        def vproj_mm(kt3, l4):
            xks = xtiles["kv", kt3]
            ltv = kt3 * 3 + l4
            psv = psS.tile([P, 2, 512], f32, name="ps_s", tag="ps_s")
            for dc in range(8):
                nc.tensor.matmul(
                    psv[:, 0, :256],
                    lhsT=xks[:, dc, ts(l4, P)],
                    rhs=wvT_sb[:, dc, :],
                    start=(dc == 0), stop=(dc == 7),
                )
            nc.vector.tensor_copy(
                V_sb[:, ltv, :, 0:64],
                psv[:, 0, :256].rearrange("p (h c) -> p h c", c=64),
            )

        # ---- attention stream ------------------------------------------
        avs = {}        # qt -> [4] PSUM accumulators [65, 512]
        avcs = {}       # qt -> [4] SBUF f16 copies
        ex1s = {}       # u -> merged [P, 3, 4, 512] exp tile
        ex2s = {}       # u -> merged [P, 3, 4, 512] weight tile
        btws = {}       # u -> bias tile
        otls = {}       # qt -> [P, 2, 512] f16 normalized O_T

        def ka_mm(ps):
            # cheap N=128 garbage matmul overwritten by the real producer
            # (start=True resets the accumulation); exists to deny the HAM
            # clock gate a contiguous idle window (keeps the PE at full rate).
            nc.tensor.matmul(ps[:, 0:128], lhsT=dummy_sb[:, 0:128],
                             rhs=dummy_sb[:, 0:128], start=True, stop=True)

        def emit_scores(qt, kt, pair, ka=0):
            swide = psS.tile([P, 2, 512], f32, name="ps_s", tag="ps_s")
            for _ in range(ka):
                ka_mm(swide[:, 0, :])
            for hi in range(2):
                cs = slice(64 * hi, 64 * hi + 64)
                nc.tensor.matmul(
                    swide[:, hi, :],
                    lhsT=kT_sb[cs, pair, ts(kt, P)],
                    rhs=qT_sb[cs, pair, ts(qt, 512)],
                    start=True, stop=True,
                )
            nc.scalar.activation(
                ex1s["cur"][:, kt % 3, ds(2 * pair, 2), :], swide, Act.Exp,
                bias=maskT_sb[:, kt:kt + 1], scale=1.0)

        def emit_mult(u, kt):
            # ex2 = ex1 * exp(bias): one merged op per kt (both pairs)
            nc.vector.tensor_mul(
                ex2s[u][:, kt % 3, :, :],
                ex1s["cur"][:, kt % 3, :, :],
                btws[u][:, kt % 3, :, :])

        def emit_av_block(w, j):
            wqt, wkt3 = divmod(w, KT3)
            if wkt3 == 0 and j == 0:
                avs[wqt] = [psAV.tile([65, 512], f32, name=f"av{wqt}{h}",
                                      tag="av") for h in range(4)]
            kt = 3 * wkt3 + j
            for h in range(4):
                nc.tensor.matmul(
                    avs[wqt][h],
                    lhsT=V_sb[:, kt, h, :],
                    rhs=ex2s[w][:, j, h, :],
                    start=(kt == 0), stop=(kt == KT - 1),
                )
            if j == 2:
                ex2s.pop(w)

        def emit_evac(qt, split=False):
            avcs[qt] = []
            for h in range(4):
                avc = normp.tile([65, 512], f16, name="avc", tag="avc")
                if split and h < 2:
                    nc.scalar.copy(avc, avs[qt][h])
                else:
                    nc.vector.tensor_copy(avc, avs[qt][h])
                avcs[qt].append(avc)

        def norm_cc(qt, cc):
            # heads {2cc, 2cc+1} -> otl[:, cc]; even head on partitions
            # 0:64, odd shifted to 64:128 via SBUF->SBUF DMA.
            e, o = avcs[qt][2 * cc], avcs[qt][2 * cc + 1]
            otl = otls[qt]
            zb = psS.tile([P, 2, 512], f32, name="ps_s", tag="ps_s")
            ka_mm(zb[:, 0, :])
            ka_mm(zb[:, 0, :])
            nc.tensor.matmul(zb[0:64, 0, :], lhsT=ones_sb[64:65, :],
                             rhs=e[64:65, :], start=True, stop=True)
            nc.tensor.matmul(zb[64:128, 0, :], lhsT=ones_sb[64:65, :],
                             rhs=o[64:65, :], start=True, stop=True)
            zr = normp.tile([P, 512], f32, name="zr", tag="zr")
            nc.vector.reciprocal_approx_fast(zr, zb[:, 0, :])
            avsh = normp.tile([P, 512], f16, name="avsh", tag="avsh")
            nc.gpsimd.dma_start(avsh[64:128, :], o[0:64, :])
            nc.vector.tensor_mul(otl[0:64, cc, :], e[0:64, :], zr[0:64, :])
            nc.vector.tensor_mul(otl[64:128, cc, :], avsh[64:128, :],
                                 zr[64:128, :])

        def oproj_mm(qt, jt, ka=0):
            ps = psS.tile([P, 2, 512], f32, name="ps_s", tag="ps_s")
            for _ in range(ka):
                ka_mm(ps[:, 0, :])
            for cc in range(2):
                nc.tensor.matmul(
                    ps[:, 0, :],
                    lhsT=woutT_sb[:, cc, ts(jt, P)],
                    rhs=otls[qt][:, cc, :],
                    start=(cc == 0), stop=(cc == 1),
                )
            if jt % 2 == 0:
                nc.scalar.copy(osbs[qt][:, jt, :], ps[:, 0, :])
            else:
                nc.vector.tensor_copy(osbs[qt][:, jt, :], ps[:, 0, :])

        osbs = {}

        def oproj_store(qt, half, quarters=2):
            w = 4 // quarters
            for q4 in range(quarters):
                o0 = 4 * half + w * q4
                nc.sync.dma_start(
                    outT_d[qt][:, ds(o0, w), :],
                    osbs[qt][:, ds(o0, w), :])

        # ---- filler schedule -------------------------------------------
        # fillers[u] = list of (dma_fn | None, [mm_fn, ...]); dma issued one
        # unit ahead, mm chunks interleaved between the unit's score slots.
        def F(dma, *mms):
            return (dma, list(mms))

        def mk_norm(qt, cc):
            def go():
                if cc == 0:
                    otls[qt] = otlp.tile([P, 2, 512], f16, name="otl",
                                         tag="otl")
                norm_cc(qt, cc)
            return go

        fillers = {u: [] for u in range(NU)}
        for kt3 in range(1, KT3):
            # kv projection for unit kt3 runs as filler in unit kt3-1
            fillers[kt3 - 1].append(F(
                (lambda k=kt3: kvproj_dma(k)),
                (lambda k=kt3: kproj_mm(k, 0)),
                (lambda k=kt3: kproj_mm(k, 1)),
                (lambda k=kt3: vproj_mm(k, 0)),
                (lambda k=kt3: vproj_mm(k, 1)),
                (lambda k=kt3: vproj_mm(k, 2)),
            ))
        for qt in range(1, 4):
            # q projection for block qt: qproj(1) late in stream 0, later
            # ones inside the (PE-thin) boundary units of streams 1 and 2
            u = 2 if qt == 1 else (qt - 1) * KT3
            fillers[u].append(F(
                (lambda l=qt: qproj_dma(l)),
                (lambda l=qt: qproj_mm(l, 0)),
                (lambda l=qt: qproj_mm(l, 1)),
            ))
        def mk_oproj(q, h):
            def go():
                if h == 0:
                    osbs[q] = outp.tile([P, 8, 512], f16, name="osb",
                                        tag="osb")
                for jt in range(4 * h, 4 * h + 4):
                    oproj_mm(q, jt, ka=1 if jt % 4 == 0 else 0)
                oproj_store(q, h)
            return go

        for qt in range(3):
            # norm of qt in the second unit of stream qt+1 (the evac is
            # emitted at the qt boundary, right after the last AV block);
            # out-proj follows in the same / next unit.
            u0 = (qt + 1) * KT3
            fillers[u0 + 1].append(F(
                None,
                mk_norm(qt, 0),
                mk_norm(qt, 1),
                mk_oproj(qt, 0),
            ))
            fillers[u0 + min(2, KT3 - 1)].append(F(None, mk_oproj(qt, 1)))

        # ---- emission ---------------------------------------------------
        # pair-0 enablers first so unit 0's scores start right after them
        qproj_mm(0, 0)
        kproj_mm(0, 0)

        for dma_fn, _ in fillers[0]:
            if dma_fn is not None:
                dma_fn()
        btw_engs = [nc.scalar, nc.gpsimd, nc.gpsimd]
        prev = None
        for u in range(NU):
            qt, kt3 = divmod(u, KT3)
            btw = biasp.tile([P, 3, HPC, 512], f16, name="btw", tag="btw")
            btw_engs[u % 3].dma_start(btw, biasT_d[u])
            btws[u] = btw
            if u + 1 < NU:
                for dma_fn, _ in fillers[u + 1]:
                    if dma_fn is not None:
                        dma_fn()
            chunks = []
            for _, mms in fillers[u]:
                chunks.extend(mms)
            ex1s["cur"] = ex1p.tile([P, 3, 4, 512], f16, name="ex1",
                                    tag="ex1")
            ex2s[u] = ex2p.tile([P, 3, 4, 512], f16, name="ex2", tag="ex2")
            if u == 0:
                # unit 0: pair-0 pass then pair-1 pass, remaining
                # projections interleaved between the slots
                u0c = [lambda: qproj_mm(0, 1), lambda: kproj_mm(0, 1),
                       lambda: vproj_mm(0, 0), lambda: vproj_mm(0, 1),
                       lambda: vproj_mm(0, 2)] + chunks
                si = 0
                for pair in range(2):
                    for kt in range(3):
                        emit_scores(0, kt, pair, ka=1 if pair == 0 else 0)
                        take = -(-(len(u0c) - si) // (6 - pair * 3 - kt))
                        for _ in range(take):
                            u0c[si]()
                            si += 1
                for kt in range(3):
                    emit_mult(0, kt)
                btws.pop(0)
                prev = 0
                continue
            pqt, pkt3 = divmod(prev, KT3)
            boundary = (kt3 == 0)
            ci = 0
            for j, kt in enumerate(range(3 * kt3, 3 * kt3 + 3)):
                ka = 1
                if j == 0 and (boundary or u in (1, 2)):
                    ka = 2
                elif u == NU - 1 and j > 0:
                    ka = 2
                emit_scores(qt, kt, 0, ka=ka)
                emit_av_block(prev, j)
                emit_scores(qt, kt, 1)
                if j == 2 and pkt3 == KT3 - 1:
                    emit_evac(pqt)
                emit_mult(u, kt)
                take = -(-(len(chunks) - ci) // (3 - j))
                for _ in range(take):
                    chunks[ci]()
                    ci += 1
            btws.pop(u)
            prev = u

        tps = psS.tile([P, 2, 512], f32, name="ps_s", tag="ps_s")
        for _ in range(20):
            ka_mm(tps[:, 0, :])
        nc.vector.tensor_copy(sink_sb[:, 0:128], tps[:, 0, 0:128])
        # tail: head-major AV so evac/norm of pair 0 overlaps pair 1's AV
        avcs[3] = [None] * 4
        otls[3] = otlp.tile([P, 2, 512], f16, name="otl", tag="otl")
        osbs[3] = outp.tile([P, 8, 512], f16, name="osb", tag="osb")
        for h in range(4):
            for j in range(3):
                kt = 3 * (KT3 - 1) + j
                nc.tensor.matmul(
                    avs[3][h],
                    lhsT=V_sb[:, kt, h, :],
                    rhs=ex2s[prev][:, j, h, :],
                    start=(kt == 0), stop=(kt == KT - 1),
                )
            avc = normp.tile([65, 512], f16, name="avc", tag="avc")
            if h < 2:
                nc.scalar.copy(avc, avs[3][h])
            else:
                nc.vector.tensor_copy(avc, avs[3][h])
            avcs[3][h] = avc
            if h == 1:
                norm_cc(3, 0)
            elif h == 3:
                norm_cc(3, 1)
        ex2s.pop(prev)
        for half in range(2):
            for jt in range(4 * half, 4 * half + 4):
                oproj_mm(3, jt, ka=1)
            oproj_store(3, half)

    nc.compile()
    return nc


def _blk(a, inner):
    """[R, C] -> [C//inner, 128, R//128, inner] device-blocked layout:
    out[ct, p, o, i] = a[o*128 + p, ct*inner + i]."""
    R, C = a.shape
    return np.ascontiguousarray(
        a.reshape(R // P, P, C // inner, inner).transpose(2, 1, 0, 3))


def _prep_core_inputs(c, Kp, x, key_padding_mask, attn_bias, W_in, b_in,
                      W_out, b_out):
    b, hg = c // HPC, c % HPC
    hs = slice(256 * hg, 256 * hg + 256)
    f16 = np.float16
    KT3 = Kp // 384
    idx = np.where(~key_padding_mask[b])[0]
    nk = len(idx)
    wq, wk = W_in[0:D][hs], W_in[D:2 * D][hs]
    wv = W_in[2 * D:3 * D][hs]

    xk = np.zeros((Kp, D), dtype=np.float32)
    xk[:nk] = x[b][idx]
    # -ln(64) headroom shift: softmax is scale-invariant, and scaling all
    # exp weights by 1/64 keeps exp(s)*exp(bias) inside fp16 range.
    maskT = np.full(Kp, -10000.0, dtype=np.float32)
    maskT[:nk] = -np.log(64.0)
    eb = np.zeros((HPC, Kp, L), dtype=f16)
    eb[:, :nk, :] = np.exp(
        attn_bias[b, HPC * hg:HPC * hg + HPC][:, :, idx]
        .transpose(0, 2, 1)).astype(f16)
    # device layout [u, p, ktm, h, q]: u = qt*KT3 + kt3,
    # key slot k = (kt3*3 + ktm)*128 + p, query l = qt*512 + q
    biasT = np.ascontiguousarray(
        eb.reshape(HPC, KT3, 3, P, 4, 512)
        .transpose(4, 1, 3, 2, 0, 5)).reshape(4 * KT3, P, 3, HPC, 512)

    xT = np.ascontiguousarray(x[b].T, dtype=f16)       # [D, L]
    xkT = np.ascontiguousarray(xk.T, dtype=f16)        # [D, Kp]
    wqkT = np.concatenate([wq / 8.0, wk], 0).T.astype(f16)   # [D, 512]
    wvT = np.ascontiguousarray(wv.T, dtype=f16)        # [D, 256]
    woutT = np.ascontiguousarray(W_out[:, hs].T, dtype=f16)  # [256, D]
    maskT_blk = np.ascontiguousarray(
        maskT.reshape(Kp // P, P).T.astype(np.float32))
    bqk = np.concatenate([b_in[0:D][hs] / 8.0,
                          b_in[D:2 * D][hs]]).astype(np.float32)
    bqk_blk = np.ascontiguousarray(bqk.reshape(4, P).T)

    return {
        "xT": _blk(xT, 512),
        "xkT": _blk(xkT, 384),
        "wqkT": _blk(wqkT, 512)[0],
        "wvT": _blk(wvT, 256)[0],
        "bqk": bqk_blk,
        "maskT": maskT_blk,
        "biasT": biasT,
        "woutT": _blk(woutT, D)[0],
    }


def kernel(x, key_padding_mask, attn_bias, W_in, b_in, W_out, b_out):
    global _compiled, LAST_RESULT
    from concourse.bass_utils import run_bass_kernel_spmd

    nk_max = int((~key_padding_mask).sum(axis=1).max())
    Kp = max(384, -(-nk_max // 384) * 384)

    if _compiled is None or _compiled[0] != Kp:
        _compiled = (Kp, _build(Kp))

    in_maps = [
        _prep_core_inputs(c, Kp, x, key_padding_mask, attn_bias, W_in, b_in,
                          W_out, b_out)
        for c in range(NCORES)
    ]
    res = run_bass_kernel_spmd(
        _compiled[1], in_maps, core_ids=list(range(NCORES)),
        trace_cores=(list(range(NCORES))
                     if os.environ.get("BASS_TRACE") == "1" else None),
    )
    LAST_RESULT = res

    # host-side epilogue: sum row-sharded partials, add biases.
    bv = b_in[2 * D:3 * D].astype(np.float64)
    const = b_out.astype(np.float64) + W_out.astype(np.float64) @ bv  # [D]
    out = np.empty((B, L, D), dtype=np.float32)
    for b in range(B):
        acc = res.results[b * HPC]["outT"].astype(np.float64)
        for g in range(1, HPC):
            acc = acc + res.results[b * HPC + g]["outT"]
        # outT blocked [qt, p, o, q] -> [D, L] -> [L, D]
        full = acc.transpose(2, 1, 0, 3).reshape(D, L)
        out[b] = (full.T + const).astype(np.float32)
    return out


# revision 27
# speedup vs baseline: 1.0834x; 1.0206x over previous
"""Biased multi-head attention on 8 Trainium2 NeuronCores.

Sharding: batch x head-group. Core c handles batch b = c//4 and heads
4*(c%4) .. 4*(c%4)+3 (4 of 16 heads). Q/K/V projections are column-sharded
over the core's heads, scores/softmax/AV are fully local per head, and the
output projection is row-sharded (each core contributes a partial [D, L]
that the host sums per batch).

Key-side compaction (the "sparse attention" lever): keys at padded
positions get score -1e4, i.e. softmax weight exp(-1e4) -> 0, so every
byte and flop spent on them is wasted. The host drops padded key
positions up front: x / attn_bias are compacted to the nk unpadded keys
(padded up to K' = ceil(nk/384)*384 slots; slack slots are re-masked with
-1e4). The result only differs from the reference by the clamped floor
weight exp(-20)/Z ~ 1e-9 the reference assigns to padded keys.

v12 design notes (~174us max-core at 2.4GHz, ~202us in the P0
2.0GHz power state, vs v2's ~212/~241):
  - All DRAM tensors are stored pre-blocked so every dma_start is 128
    descriptors of one large contiguous per-partition run (8-12KB each)
    instead of 1024-1536 x 1KB: descriptor issuance (DIRECT2D, ~15ns
    each) runs ON the issuing engine's sequencer, and the v2 layouts
    stalled the exp/score stream behind 10-27us issuance bursts.
  - AV matmul blocks of unit u-1 are interleaved BETWEEN unit u's score
    slots (after the pair-0 matmuls) instead of batched at unit end:
    the scores phase outruns the ACT exp stream (1.06us/pair-exp) and
    stalls on the 2-deep score-PSUM rotation; the AV block gives ACT
    ~1us of catch-up per slot.
  - ex2 = ex1 * exp(bias) as one merged [P, 4, 512] DVE op per kt.
  - Ramp: pair-0 q/k projections emitted first, unit 0 interleaves the
    remaining projections between its slots (pair-0 pass then pair-1
    pass); x0 loads on the SW-DGE group in parallel with wqkT on the
    HW group; 36 warm-up matmuls cover the load wait (HAM clock gate).
  - sync queue = consts + outT stores only (stores wait 3-5us on data,
    which must not head-of-line-block bias loads); bias rotates
    [scalar, gpsimd, gpsimd]; stores split in quarters; tail AV is
    head-major so evac/norm of pair 0 overlap pair 1's AV matmuls;
    20 dummy matmuls + ka on the last unit's slots bridge the final
    exp/mult drain so the tail runs at full clock. Mid-stream is NOT
    clock-bound (post-stall MMs measure 216+~106 LDW, i.e. warm): the
    residual ~17us of PE gaps is the 2-buffer score-PSUM/exp lockstep,
    which every deeper-buffering variant failed to beat (see below).

Measured dead ends (do not retry blindly): per-hi exps (either PSUM
layout) lose the score-pair row-tiling concurrency or the exp lead;
fp8 DoubleRow projections are 2x faster on PE but give ~6.6% rel err
(random-sign sums keep per-element quantization noise, it does not
average down) vs the 2e-2 gate; psS-parity padding via reader-less
tiles wedges the pool rotation; moving oproj copies off ACT or
rebalancing bias queues regresses; head-major boundary AV with
per-head evacs is neutral; removing the per-slot kas costs ~2us. NOTE: the chip lottery matters -
runs land at 2.4GHz (MM N=512 p50=216ns) or P0-throttled 2.0GHz
(p50=259, ~+20% wall); compare kernels only within the same state.
"""

import os

import numpy as np

B, L, D, H = 2, 2048, 1024, 16
dh = D // H          # 64
NCORES = 8
HPC = 4              # heads per core
P = 128

_compiled = None     # (Kp, nc): compiled module and its key-slot count
LAST_RESULT = None   # BassKernelResults of the most recent run (for profiling)


def _build(Kp):
    from contextlib import ExitStack

    import concourse.bass as bass
    import concourse.tile as tile
    from concourse import bacc, mybir
    from concourse.bass import ds, ts

    f32 = mybir.dt.float32
    f16 = mybir.dt.float16
    Act = mybir.ActivationFunctionType
    KT = Kp // P          # 128-wide key chunks
    KT3 = Kp // 384       # 384-wide key units
    NU = 4 * KT3          # stream units total

    nc = bacc.Bacc("TRN2", target_bir_lowering=False, debug=False,
                   num_devices=NCORES)

    # blocked layouts: one contiguous per-partition run per load/store
    xT_d = nc.dram_tensor("xT", [4, P, 8, 512], f16, kind="ExternalInput").ap()
    xkT_d = nc.dram_tensor("xkT", [KT3, P, 8, 384], f16,
                           kind="ExternalInput").ap()
    wqkT_d = nc.dram_tensor("wqkT", [P, 8, 512], f16,
                            kind="ExternalInput").ap()
    wvT_d = nc.dram_tensor("wvT", [P, 8, 256], f16, kind="ExternalInput").ap()
    bqk_d = nc.dram_tensor("bqk", [P, 4], f32, kind="ExternalInput").ap()
    maskT_d = nc.dram_tensor("maskT", [P, KT], f32, kind="ExternalInput").ap()
    biasT_d = nc.dram_tensor("biasT", [NU, P, 3, HPC, 512], f16,
                             kind="ExternalInput").ap()
    woutT_d = nc.dram_tensor("woutT", [P, 2, D], f16,
                             kind="ExternalInput").ap()
    outT_d = nc.dram_tensor("outT", [4, P, 8, 512], f16,
                            kind="ExternalOutput").ap()

    with tile.TileContext(nc) as tc, ExitStack() as ctx:
        consts = ctx.enter_context(tc.tile_pool(name="consts", bufs=1))
        xp = ctx.enter_context(tc.tile_pool(name="xp", bufs=2))
        xkp = ctx.enter_context(tc.tile_pool(name="xkp", bufs=2))
        biasp = ctx.enter_context(tc.tile_pool(name="biasp", bufs=3))
        ex1p = ctx.enter_context(tc.tile_pool(name="ex1p", bufs=2))
        ex2p = ctx.enter_context(tc.tile_pool(name="ex2p", bufs=2))
        normp = ctx.enter_context(tc.tile_pool(name="normp", bufs=5))
        outp = ctx.enter_context(tc.tile_pool(name="outp", bufs=2))
        otlp = ctx.enter_context(tc.tile_pool(name="otlp", bufs=2))
        psS = ctx.enter_context(tc.tile_pool(name="psS", bufs=2, space="PSUM"))
        psAV = ctx.enter_context(tc.tile_pool(name="psAV", bufs=4, space="PSUM"))

        # criticality-ordered initial loads, split across both DGE groups:
        # HW (sync, scalar): wqkT + x0 + xk0 + small consts the first exps
        # and evacs need; SW (gpsimd): V-projection weights and the rest.
        wqkT_sb = consts.tile([P, 8, 512], f16, name="wqkT_sb", tag="wqkT_sb")
        nc.sync.dma_start(wqkT_sb, wqkT_d)

        xtiles = {}

        def qproj_dma(lt, eng=None):
            xs = xp.tile([P, 8, 512], f16, name="xs", tag="xs")
            (eng or nc.gpsimd).dma_start(xs, xT_d[lt])
            xtiles["q", lt] = xs

        def kvproj_dma(kt3, eng=None):
            xks = xkp.tile([P, 8, 384], f16, name="xks", tag="xks")
            (eng or nc.gpsimd).dma_start(xks, xkT_d[kt3])
            xtiles["kv", kt3] = xks

        qproj_dma(0)
        kvproj_dma(0, nc.scalar)
        maskT_sb = consts.tile([P, KT], f32, name="maskT_sb", tag="maskT_sb")
        nc.sync.dma_start(maskT_sb, maskT_d)
        bqk_sb = consts.tile([P, 4], f32, name="bqk_sb", tag="bqk_sb")
        nc.scalar.dma_start(bqk_sb, bqk_d)
        wvT_sb = consts.tile([P, 8, 256], f16, name="wvT_sb", tag="wvT_sb")
        nc.gpsimd.dma_start(wvT_sb, wvT_d)
        woutT_sb = consts.tile([P, 2, D], f16, name="woutT_sb", tag="woutT_sb")
        nc.gpsimd.dma_start(woutT_sb, woutT_d)

        qT_sb = consts.tile([P, 2, L], f16, name="qT_sb", tag="qT_sb")
        kT_sb = consts.tile([P, 2, Kp], f16, name="kT_sb", tag="kT_sb")
        V_sb = consts.tile([P, KT, HPC, 65], f16, name="V_sb", tag="V_sb")

        ones_sb = consts.tile([P, 64], f16, name="ones_sb", tag="ones_sb")
        nc.vector.memset(ones_sb, 1.0)
        nc.vector.tensor_copy(
            V_sb[:, :, :, 64:65],
            ones_sb[:, 0:1, None, None].to_broadcast((P, KT, HPC, 1)),
        )

        # HAM warm-up: ~9us of dependency-free matmuls at t=0 (covering the
        # initial DMA wait) so the PE clock gate is at full rate when the
        # first projection matmuls arrive.
        dummy_sb = consts.tile([P, 512], f16, name="dummy_sb", tag="dummy_sb")
        nc.vector.memset(dummy_sb, 0.5)
        sink_sb = consts.tile([P, 512], f16, name="sink_sb", tag="sink_sb")
        dps = psS.tile([P, 2, 512], f32, name="ps_s", tag="ps_s")
        for i in range(36):
            nc.tensor.matmul(dps[:, 0, :], lhsT=dummy_sb[:, 0:128],
                             rhs=dummy_sb, start=(i == 0), stop=(i == 35))
        nc.vector.tensor_copy(sink_sb, dps[:, 0, :])
        # pre-load the ACT exp table before the stream needs it
        nc.scalar.activation(sink_sb[:, 0:1], dummy_sb[:, 0:1],
                             Act.Exp, scale=1.0)

        # ---- projections (dma / mm chunks separated for scheduling) ----
        def qproj_mm(lt, rt):
            ps = psS.tile([P, 2, 512], f32, name="ps_s", tag="ps_s")
            for dc in range(8):
                nc.tensor.matmul(
                    ps[:, 0, :],
                    lhsT=wqkT_sb[:, dc, ts(rt, P)],
                    rhs=xtiles["q", lt][:, dc, :],
                    start=(dc == 0), stop=(dc == 7),
                )
            nc.vector.tensor_scalar_add(
                qT_sb[:, rt, ts(lt, 512)], ps[:, 0, :], bqk_sb[:, rt:rt + 1])

        def kproj_mm(kt3, rt):
            xks = xtiles["kv", kt3]
            ps = psS.tile([P, 2, 512], f32, name="ps_s", tag="ps_s")
            for dc in range(8):
                nc.tensor.matmul(
                    ps[:, 0, :384],
                    lhsT=wqkT_sb[:, dc, ts(2 + rt, P)],
                    rhs=xks[:, dc, :],
                    start=(dc == 0), stop=(dc == 7),
                )
            nc.vector.tensor_scalar_add(
                kT_sb[:, rt, ts(kt3, 384)], ps[:, 0, :384],
                bqk_sb[:, 2 + rt:3 + rt])

        def vproj_mm(kt3, l4):
            xks = xtiles["kv", kt3]
            ltv = kt3 * 3 + l4
            psv = psS.tile([P, 2, 512], f32, name="ps_s", tag="ps_s")
            for dc in range(8):
                nc.tensor.matmul(
                    psv[:, 0, :256],
                    lhsT=xks[:, dc, ts(l4, P)],
                    rhs=wvT_sb[:, dc, :],
                    start=(dc == 0), stop=(dc == 7),
                )
            nc.vector.tensor_copy(
                V_sb[:, ltv, :, 0:64],
                psv[:, 0, :256].rearrange("p (h c) -> p h c", c=64),
            )

        # ---- attention stream ------------------------------------------
        avs = {}        # qt -> [4] PSUM accumulators [65, 512]
        avcs = {}       # qt -> [4] SBUF f16 copies
        ex1s = {}       # u -> merged [P, 3, 4, 512] exp tile
        ex2s = {}       # u -> merged [P, 3, 4, 512] weight tile
        btws = {}       # u -> bias tile
        otls = {}       # qt -> [P, 2, 512] f16 normalized O_T

        def ka_mm(ps):
            # cheap N=128 garbage matmul overwritten by the real producer
            # (start=True resets the accumulation); exists to deny the HAM
            # clock gate a contiguous idle window (keeps the PE at full rate).
            nc.tensor.matmul(ps[:, 0:128], lhsT=dummy_sb[:, 0:128],
                             rhs=dummy_sb[:, 0:128], start=True, stop=True)

        def emit_scores(qt, kt, pair, ka=0):
            swide = psS.tile([P, 2, 512], f32, name="ps_s", tag="ps_s")
            for _ in range(ka):
                ka_mm(swide[:, 0, :])
            for hi in range(2):
                cs = slice(64 * hi, 64 * hi + 64)
                nc.tensor.matmul(
                    swide[:, hi, :],
                    lhsT=kT_sb[cs, pair, ts(kt, P)],
                    rhs=qT_sb[cs, pair, ts(qt, 512)],
                    start=True, stop=True,
                )
            nc.scalar.activation(
                ex1s["cur"][:, kt % 3, ds(2 * pair, 2), :], swide, Act.Exp,
                bias=maskT_sb[:, kt:kt + 1], scale=1.0)

        def emit_mult(u, kt):
            # ex2 = ex1 * exp(bias): one merged op per kt (both pairs)
            nc.vector.tensor_mul(
                ex2s[u][:, kt % 3, :, :],
                ex1s["cur"][:, kt % 3, :, :],
                btws[u][:, kt % 3, :, :])

        def emit_av_block(w, j):
            wqt, wkt3 = divmod(w, KT3)
            if wkt3 == 0 and j == 0:
                avs[wqt] = [psAV.tile([65, 512], f32, name=f"av{wqt}{h}",
                                      tag="av") for h in range(4)]
            kt = 3 * wkt3 + j
            for h in range(4):
                nc.tensor.matmul(
                    avs[wqt][h],
                    lhsT=V_sb[:, kt, h, :],
                    rhs=ex2s[w][:, j, h, :],
                    start=(kt == 0), stop=(kt == KT - 1),
                )
            if j == 2:
                ex2s.pop(w)

        def emit_evac(qt, split=False):
            avcs[qt] = []
            for h in range(4):
                avc = normp.tile([65, 512], f16, name="avc", tag="avc")
                if split and h < 2:
                    nc.scalar.copy(avc, avs[qt][h])
                else:
                    nc.vector.tensor_copy(avc, avs[qt][h])
                avcs[qt].append(avc)

        def norm_cc(qt, cc):
            # heads {2cc, 2cc+1} -> otl[:, cc]; even head on partitions
            # 0:64, odd shifted to 64:128 via SBUF->SBUF DMA.
            e, o = avcs[qt][2 * cc], avcs[qt][2 * cc + 1]
            otl = otls[qt]
            zb = psS.tile([P, 2, 512], f32, name="ps_s", tag="ps_s")
            ka_mm(zb[:, 0, :])
            ka_mm(zb[:, 0, :])
            nc.tensor.matmul(zb[0:64, 0, :], lhsT=ones_sb[64:65, :],
                             rhs=e[64:65, :], start=True, stop=True)
            nc.tensor.matmul(zb[64:128, 0, :], lhsT=ones_sb[64:65, :],
                             rhs=o[64:65, :], start=True, stop=True)
            zr = normp.tile([P, 512], f32, name="zr", tag="zr")
            nc.vector.reciprocal_approx_fast(zr, zb[:, 0, :])
            avsh = normp.tile([P, 512], f16, name="avsh", tag="avsh")
            nc.gpsimd.dma_start(avsh[64:128, :], o[0:64, :])
            nc.vector.tensor_mul(otl[0:64, cc, :], e[0:64, :], zr[0:64, :])
            nc.vector.tensor_mul(otl[64:128, cc, :], avsh[64:128, :],
                                 zr[64:128, :])

        def oproj_mm(qt, jt, ka=0):
            ps = psS.tile([P, 2, 512], f32, name="ps_s", tag="ps_s")
            for _ in range(ka):
                ka_mm(ps[:, 0, :])
            for cc in range(2):
                nc.tensor.matmul(
                    ps[:, 0, :],
                    lhsT=woutT_sb[:, cc, ts(jt, P)],
                    rhs=otls[qt][:, cc, :],
                    start=(cc == 0), stop=(cc == 1),
                )
            if jt % 2 == 0:
                nc.scalar.copy(osbs[qt][:, jt, :], ps[:, 0, :])
            else:
                nc.vector.tensor_copy(osbs[qt][:, jt, :], ps[:, 0, :])

        osbs = {}

        def oproj_store(qt, half, quarters=2):
            w = 4 // quarters
            for q4 in range(quarters):
                o0 = 4 * half + w * q4
                nc.sync.dma_start(
                    outT_d[qt][:, ds(o0, w), :],
                    osbs[qt][:, ds(o0, w), :])

        # ---- filler schedule -------------------------------------------
        # fillers[u] = list of (dma_fn | None, [mm_fn, ...]); dma issued one
        # unit ahead, mm chunks interleaved between the unit's score slots.
        def F(dma, *mms):
            return (dma, list(mms))

        def mk_norm(qt, cc):
            def go():
                if cc == 0:
                    otls[qt] = otlp.tile([P, 2, 512], f16, name="otl",
                                         tag="otl")
                norm_cc(qt, cc)
            return go

        fillers = {u: [] for u in range(NU)}
        for kt3 in range(1, KT3):
            # kv projection for unit kt3 runs as filler in unit kt3-1
            fillers[kt3 - 1].append(F(
                (lambda k=kt3: kvproj_dma(k)),
                (lambda k=kt3: kproj_mm(k, 0)),
                (lambda k=kt3: kproj_mm(k, 1)),
                (lambda k=kt3: vproj_mm(k, 0)),
                (lambda k=kt3: vproj_mm(k, 1)),
                (lambda k=kt3: vproj_mm(k, 2)),
            ))
        for qt in range(1, 4):
            # q projection for block qt: qproj(1) late in stream 0, later
            # ones inside the (PE-thin) boundary units of streams 1 and 2
            u = 2 if qt == 1 else (qt - 1) * KT3
            fillers[u].append(F(
                (lambda l=qt: qproj_dma(l)),
                (lambda l=qt: qproj_mm(l, 0)),
                (lambda l=qt: qproj_mm(l, 1)),
            ))
        def mk_oproj(q, h):
            def go():
                if h == 0:
                    osbs[q] = outp.tile([P, 8, 512], f16, name="osb",
                                        tag="osb")
                for jt in range(4 * h, 4 * h + 4):
                    oproj_mm(q, jt, ka=1 if jt % 4 == 0 else 0)
                oproj_store(q, h)
            return go

        for qt in range(3):
            # norm of qt in the second unit of stream qt+1 (the evac is
            # emitted at the qt boundary, right after the last AV block);
            # out-proj follows in the same / next unit.
            u0 = (qt + 1) * KT3
            fillers[u0 + 1].append(F(
                None,
                mk_norm(qt, 0),
                mk_norm(qt, 1),
                mk_oproj(qt, 0),
            ))
            fillers[u0 + min(2, KT3 - 1)].append(F(None, mk_oproj(qt, 1)))

        # ---- emission ---------------------------------------------------
        # pair-0 enablers first so unit 0's scores start right after them
        qproj_mm(0, 0)
        kproj_mm(0, 0)

        for dma_fn, _ in fillers[0]:
            if dma_fn is not None:
                dma_fn()
        btw_engs = [nc.scalar, nc.gpsimd, nc.gpsimd]
        prev = None
        for u in range(NU):
            qt, kt3 = divmod(u, KT3)
            btw = biasp.tile([P, 3, HPC, 512], f16, name="btw", tag="btw")
            btw_engs[u % 3].dma_start(btw, biasT_d[u])
            btws[u] = btw
            if u + 1 < NU:
                for dma_fn, _ in fillers[u + 1]:
                    if dma_fn is not None:
                        dma_fn()
            chunks = []
            for _, mms in fillers[u]:
                chunks.extend(mms)
            ex1s["cur"] = ex1p.tile([P, 3, 4, 512], f16, name="ex1",
                                    tag="ex1")
            ex2s[u] = ex2p.tile([P, 3, 4, 512], f16, name="ex2", tag="ex2")
            if u == 0:
                # unit 0: pair-0 pass then pair-1 pass, remaining
                # projections interleaved between the slots
                u0c = [lambda: qproj_mm(0, 1), lambda: kproj_mm(0, 1),
                       lambda: vproj_mm(0, 0), lambda: vproj_mm(0, 1),
                       lambda: vproj_mm(0, 2)] + chunks
                si = 0
                for pair in range(2):
                    for kt in range(3):
                        emit_scores(0, kt, pair, ka=1 if pair == 0 else 0)
                        take = -(-(len(u0c) - si) // (6 - pair * 3 - kt))
                        for _ in range(take):
                            u0c[si]()
                            si += 1
                for kt in range(3):
                    emit_mult(0, kt)
                btws.pop(0)
                prev = 0
                continue
            pqt, pkt3 = divmod(prev, KT3)
            boundary = (kt3 == 0)
            ci = 0
            for j, kt in enumerate(range(3 * kt3, 3 * kt3 + 3)):
                ka = 1
                if j == 0 and (boundary or u in (1, 2)):
                    ka = 2
                elif u == NU - 1 and j > 0:
                    ka = 2
                emit_scores(qt, kt, 0, ka=ka)
                emit_av_block(prev, j)
                emit_scores(qt, kt, 1)
                if j == 2 and pkt3 == KT3 - 1:
                    emit_evac(pqt)
                emit_mult(u, kt)
                take = -(-(len(chunks) - ci) // (3 - j))
                for _ in range(take):
                    chunks[ci]()
                    ci += 1
            btws.pop(u)
            prev = u

        tps = psS.tile([P, 2, 512], f32, name="ps_s", tag="ps_s")
        for _ in range(20):
            ka_mm(tps[:, 0, :])
        nc.vector.tensor_copy(sink_sb[:, 0:128], tps[:, 0, 0:128])
        # tail: head-major AV so evac/norm of pair 0 overlaps pair 1's AV
        avcs[3] = [None] * 4
        otls[3] = otlp.tile([P, 2, 512], f16, name="otl", tag="otl")
        osbs[3] = outp.tile([P, 8, 512], f16, name="osb", tag="osb")
        for h in range(4):
            for j in range(3):
                kt = 3 * (KT3 - 1) + j
                nc.tensor.matmul(
                    avs[3][h],
                    lhsT=V_sb[:, kt, h, :],
                    rhs=ex2s[prev][:, j, h, :],
                    start=(kt == 0), stop=(kt == KT - 1),
                )
            avc = normp.tile([65, 512], f16, name="avc", tag="avc")
            if h < 2:
                nc.scalar.copy(avc, avs[3][h])
            else:
                nc.vector.tensor_copy(avc, avs[3][h])
            avcs[3][h] = avc
            if h == 1:
                norm_cc(3, 0)
            elif h == 3:
                norm_cc(3, 1)
        ex2s.pop(prev)
        for half in range(2):
            for jt in range(4 * half, 4 * half + 4):
                oproj_mm(3, jt, ka=1)
            oproj_store(3, half)

    nc.compile()
    return nc


def _blk(a, inner):
    """[R, C] -> [C//inner, 128, R//128, inner] device-blocked layout:
    out[ct, p, o, i] = a[o*128 + p, ct*inner + i]."""
    R, C = a.shape
    return np.ascontiguousarray(
        a.reshape(R // P, P, C // inner, inner).transpose(2, 1, 0, 3))


def _prep_core_inputs(c, Kp, x, key_padding_mask, attn_bias, W_in, b_in,
                      W_out, b_out):
    b, hg = c // HPC, c % HPC
    hs = slice(256 * hg, 256 * hg + 256)
    f16 = np.float16
    KT3 = Kp // 384
    idx = np.where(~key_padding_mask[b])[0]
    nk = len(idx)
    wq, wk = W_in[0:D][hs], W_in[D:2 * D][hs]
    wv = W_in[2 * D:3 * D][hs]

    xk = np.zeros((Kp, D), dtype=np.float32)
    xk[:nk] = x[b][idx]
    # -ln(64) headroom shift: softmax is scale-invariant, and scaling all
    # exp weights by 1/64 keeps exp(s)*exp(bias) inside fp16 range.
    maskT = np.full(Kp, -10000.0, dtype=np.float32)
    maskT[:nk] = -np.log(64.0)
    eb = np.zeros((HPC, Kp, L), dtype=f16)
    eb[:, :nk, :] = np.exp(
        attn_bias[b, HPC * hg:HPC * hg + HPC][:, :, idx]
        .transpose(0, 2, 1)).astype(f16)
    # device layout [u, p, ktm, h, q]: u = qt*KT3 + kt3,
    # key slot k = (kt3*3 + ktm)*128 + p, query l = qt*512 + q
    biasT = np.ascontiguousarray(
        eb.reshape(HPC, KT3, 3, P, 4, 512)
        .transpose(4, 1, 3, 2, 0, 5)).reshape(4 * KT3, P, 3, HPC, 512)

    xT = np.ascontiguousarray(x[b].T, dtype=f16)       # [D, L]
    xkT = np.ascontiguousarray(xk.T, dtype=f16)        # [D, Kp]
    wqkT = np.concatenate([wq / 8.0, wk], 0).T.astype(f16)   # [D, 512]
    wvT = np.ascontiguousarray(wv.T, dtype=f16)        # [D, 256]
    woutT = np.ascontiguousarray(W_out[:, hs].T, dtype=f16)  # [256, D]
    maskT_blk = np.ascontiguousarray(
        maskT.reshape(Kp // P, P).T.astype(np.float32))
    bqk = np.concatenate([b_in[0:D][hs] / 8.0,
                          b_in[D:2 * D][hs]]).astype(np.float32)
    bqk_blk = np.ascontiguousarray(bqk.reshape(4, P).T)

    return {
        "xT": _blk(xT, 512),
        "xkT": _blk(xkT, 384),
        "wqkT": _blk(wqkT, 512)[0],
        "wvT": _blk(wvT, 256)[0],
        "bqk": bqk_blk,
        "maskT": maskT_blk,
        "biasT": biasT,
        "woutT": _blk(woutT, D)[0],
    }


def kernel(x, key_padding_mask, attn_bias, W_in, b_in, W_out, b_out):
    global _compiled, LAST_RESULT
    from concourse.bass_utils import run_bass_kernel_spmd

    nk_max = int((~key_padding_mask).sum(axis=1).max())
    Kp = max(384, -(-nk_max // 384) * 384)

    if _compiled is None or _compiled[0] != Kp:
        _compiled = (Kp, _build(Kp))

    in_maps = [
        _prep_core_inputs(c, Kp, x, key_padding_mask, attn_bias, W_in, b_in,
                          W_out, b_out)
        for c in range(NCORES)
    ]
    res = run_bass_kernel_spmd(
        _compiled[1], in_maps, core_ids=list(range(NCORES)),
        trace_cores=(list(range(NCORES))
                     if os.environ.get("BASS_TRACE") == "1" else None),
    )
    LAST_RESULT = res

    # host-side epilogue: sum row-sharded partials, add biases.
    bv = b_in[2 * D:3 * D].astype(np.float64)
    const = b_out.astype(np.float64) + W_out.astype(np.float64) @ bv  # [D]
    out = np.empty((B, L, D), dtype=np.float32)
    for b in range(B):
        acc = res.results[b * HPC]["outT"].astype(np.float64)
        for g in range(1, HPC):
            acc = acc + res.results[b * HPC + g]["outT"]
        # outT blocked [qt, p, o, q] -> [D, L] -> [L, D]
        full = acc.transpose(2, 1, 0, 3).reshape(D, L)
        out[b] = (full.T + const).astype(np.float32)
    return out
